# revision 1
# baseline (speedup 1.0000x reference)
"""MiniTransformerLayer on 8 Trainium2 NeuronCores.

Sharding (single kernel launch, 2 collectives, no all-reduce):
  - tokens t = b*S + s flattened to [4096]; core c owns tokens [512c, 512(c+1))
    and heads {2c, 2c+1} (for both batches).
  - LN1 computed on own token shard (activations kept transposed [hidden, token]),
    AllGather -> full h^T on every core.
  - qkv column-sharded by head. q,k produced feature-major [d, t] (with an
    even/odd d-permutation so RoPE needs no partition swaps), v token-major [t, d].
  - attention per (batch, head): scores computed transposed (s^T[k,q] = k^T.T @ q^T),
    exp on ScalarE (constant -3 bias instead of row-max; cancels in normalization),
    denominator = ones-vector matmul over a DVE-folded chunk accumulator,
    attn@V contracts k directly with p^T as the moving operand -> out [d, q].
  - AllToAll converts head-sharded attn output to token-sharded full-feature.
  - out_proj / MLP computed data-parallel on own 512 tokens with replicated
    (streamed) weights. Residual path in fp32; matmul operands fp16.
"""

import sys

sys.path.insert(0, "/opt/trn_rl_repo")

import numpy as np

import concourse.bass as bass
import concourse.bacc as bacc
import concourse.tile as tile
import concourse.mybir as mybir
from concourse import bass_utils

F16 = mybir.dt.float16
F32 = mybir.dt.float32
AF = mybir.ActivationFunctionType

NCORES = 8
B, S, HID, HEADS, D, FFN = 2, 2048, 2048, 16, 128, 4096
TOK = B * S            # 4096 flat tokens
TPC = TOK // NCORES    # 512 tokens per core
HC = HID // 128        # 16 hidden chunks
FFC = FFN // 128       # 32 ffn chunks
NH = HEADS // NCORES   # 2 heads per core
SCALE = 1.0 / float(np.sqrt(D))
EXP_BIAS = -3.0
EPS = 1e-5

_CACHE = {}


def _emit(nc, single_core=False):
    xT = nc.dram_tensor("xT", [HID, TPC], F32, kind="ExternalInput")
    wq = nc.dram_tensor("wq", [128, HC * NH * 128], F16, kind="ExternalInput")
    wk = nc.dram_tensor("wk", [128, HC * NH * 128], F16, kind="ExternalInput")
    wv = nc.dram_tensor("wv", [128, HC * NH * 128], F16, kind="ExternalInput")
    wo = nc.dram_tensor("wo", [HC * 128, HC * 128], F16, kind="ExternalInput")
    wf1 = nc.dram_tensor("wf1", [FFC * 128, HC * 128], F16, kind="ExternalInput")
    wf2 = nc.dram_tensor("wf2", [HC * 128, FFC * 128], F16, kind="ExternalInput")
    g1 = nc.dram_tensor("g1", [128, HC], F32, kind="ExternalInput")
    b1 = nc.dram_tensor("b1", [128, HC], F32, kind="ExternalInput")
    g2 = nc.dram_tensor("g2", [128, HC], F32, kind="ExternalInput")
    b2 = nc.dram_tensor("b2", [128, HC], F32, kind="ExternalInput")
    ropeC = nc.dram_tensor("ropeC", [128, TOK], F16, kind="ExternalInput")
    ropeS = nc.dram_tensor("ropeS", [128, TOK], F16, kind="ExternalInput")
    outT = nc.dram_tensor("outT", [HID, TPC], F32, kind="ExternalOutput")

    rg = [list(range(NCORES))]
    MULT, ADD = mybir.AluOpType.mult, mybir.AluOpType.add

    with tile.TileContext(nc) as tc:
        with (
            tc.tile_pool(name="const", bufs=1) as const,
            tc.tile_pool(name="dram", bufs=1, space="DRAM") as dram,
        ):
            ones_col = const.tile([128, 1], F32, tag="onc")
            nc.vector.memset(ones_col[:], 1.0)
            ones_col16 = const.tile([128, 1], F16, tag="onc16")
            nc.vector.memset(ones_col16[:], 1.0)
            ones_row = const.tile([1, 128], F32, tag="onr")
            nc.vector.memset(ones_row[:], 1.0)
            eps_b = const.tile([1, 1], F32, tag="epsb")
            nc.vector.memset(eps_b[:], EPS)
            zero1_b = const.tile([1, 1], F32, tag="z1b")
            nc.vector.memset(zero1_b[:], 0.0)
            zero_b = const.tile([128, 1], F32, tag="zb")
            nc.vector.memset(zero_b[:], 0.0)
            expb_b = const.tile([128, 1], F32, tag="expb")
            nc.vector.memset(expb_b[:], EXP_BIAS)
            g1_sb = const.tile([128, HC], F32, tag="g1")
            b1_sb = const.tile([128, HC], F32, tag="b1")
            g2_sb = const.tile([128, HC], F32, tag="g2")
            b2_sb = const.tile([128, HC], F32, tag="b2")
            nc.scalar.dma_start(g1_sb[:], g1[:])
            nc.scalar.dma_start(b1_sb[:], b1[:])
            nc.scalar.dma_start(g2_sb[:], g2[:])
            nc.scalar.dma_start(b2_sb[:], b2[:])

            ag_in_a = dram.tile([HID // 2, TPC], F16)
            ag_in_b = dram.tile([HID // 2, TPC], F16)
            a2a_in_m = [dram.tile([NCORES * 128, TPC], F16, name=f"a2ai{m}")
                        for m in range(NH)]
            a2a_out_m = [dram.tile([NCORES * 128, TPC], F16,
                                   name=f"a2ao{m}") for m in range(NH)]
            if single_core:
                ag_out_a = dram.tile([NCORES * HID // 2, TPC], F16)
                ag_out_b = dram.tile([NCORES * HID // 2, TPC], F16)
            else:
                ag_out_a = nc.dram_tensor(
                    "ag_out_a_sh", [NCORES * HID // 2, TPC], F16,
                    addr_space="Shared").ap()
                ag_out_b = nc.dram_tensor(
                    "ag_out_b_sh", [NCORES * HID // 2, TPC], F16,
                    addr_space="Shared").ap()

            def layernorm(get_src, put_dst, gg, bb, lnp, psst, psbc):
                # h = (x - mu) * rstd * g + b, contraction over partitions via
                # ones-matmuls; per-token coeffs broadcast via K=1 matmuls.
                ps_sx = psst.tile([1, TPC], F32, tag="st")
                ps_sq = psst.tile([1, TPC], F32, tag="st")
                for j in range(HC):
                    s = get_src(j)
                    sqt = lnp.tile([128, TPC], F32, tag="sqt")
                    nc.vector.tensor_mul(sqt[:], s, s)
                    nc.tensor.matmul(ps_sx[:], ones_col[:], s,
                                     start=(j == 0), stop=(j == HC - 1))
                    nc.tensor.matmul(ps_sq[:], ones_col[:], sqt[:],
                                     start=(j == 0), stop=(j == HC - 1))
                mu = lnp.tile([1, TPC], F32, tag="mu")
                m2 = lnp.tile([1, TPC], F32, tag="m2")
                var = lnp.tile([1, TPC], F32, tag="var")
                lnv = lnp.tile([1, TPC], F32, tag="lnv")
                rstd = lnp.tile([1, TPC], F32, tag="rstd")
                mrs = lnp.tile([1, TPC], F32, tag="mrs")
                nc.vector.tensor_scalar_mul(mu[:], ps_sx[:], 1.0 / HID)
                nc.vector.tensor_scalar_mul(m2[:], ps_sq[:], 1.0 / HID)
                nc.vector.tensor_mul(var[:], mu[:], mu[:])
                nc.vector.tensor_sub(var[:], m2[:], var[:])
                nc.scalar.activation(lnv[:], var[:], AF.Ln, bias=eps_b[:])
                nc.scalar.activation(rstd[:], lnv[:], AF.Exp, bias=zero1_b[:],
                                     scale=-0.5)
                nc.vector.tensor_mul(mrs[:], mu[:], rstd[:])
                nc.vector.tensor_scalar_mul(mrs[:], mrs[:], -1.0)
                ps_c1 = psbc.tile([128, TPC], F32, tag="bc")
                ps_c0 = psbc.tile([128, TPC], F32, tag="bc")
                nc.tensor.matmul(ps_c1[:], ones_row[:], rstd[:], start=True, stop=True)
                nc.tensor.matmul(ps_c0[:], ones_row[:], mrs[:], start=True, stop=True)
                for j in range(HC):
                    s = get_src(j)
                    t1 = lnp.tile([128, TPC], F32, tag="t1")
                    t2 = lnp.tile([128, TPC], F32, tag="t2")
                    nc.vector.tensor_mul(t1[:], s, ps_c1[:])
                    nc.vector.tensor_add(t2[:], t1[:], ps_c0[:])
                    put_dst(j, t2, gg[:, j:j + 1], bb[:, j:j + 1])

            # ---------------- Stage A: LN1 (x streamed) + AllGather ----------
            with (
                tc.tile_pool(name="lnA", bufs=3) as lnA,
                tc.tile_pool(name="psstA", bufs=2, space="PSUM") as psstA,
                tc.tile_pool(name="psbcA", bufs=2, space="PSUM") as psbcA,
            ):
                def get_x(j):
                    t = lnA.tile([128, TPC], F32, tag="xs")
                    nc.sync.dma_start(t[:], xT[j * 128:(j + 1) * 128, :])
                    return t[:]

                def put_h1(j, t2, gj, bj):
                    hc_t = lnA.tile([128, TPC], F16, tag="hc")
                    nc.gpsimd.tensor_scalar(hc_t[:], t2[:], gj, bj, MULT, ADD)
                    tgt = ag_in_a if j < 8 else ag_in_b
                    jj = j % 8
                    nc.sync.dma_start(tgt[jj * 128:(jj + 1) * 128, :], hc_t[:])

                layernorm(get_x, put_h1, g1_sb, b1_sb, lnA, psstA, psbcA)

            H2 = HID // 2
            if single_core:
                # timing stand-in for AllGather (~real collective cost): one
                # quarter-width write per rank slot establishes deps + ~15us
                for r in range(NCORES):
                    nc.sync.dma_start(ag_out_a[r * H2:(r + 1) * H2, 0:TPC // 4],
                                      ag_in_a[:, 0:TPC // 4])
                    nc.sync.dma_start(ag_out_b[r * H2:(r + 1) * H2, 0:TPC // 4],
                                      ag_in_b[:, 0:TPC // 4])
            else:
                nc.gpsimd.collective_compute(
                    "AllGather", mybir.AluOpType.bypass, replica_groups=rg,
                    ins=[ag_in_a.opt()], outs=[ag_out_a],
                )
                nc.gpsimd.collective_compute(
                    "AllGather", mybir.AluOpType.bypass, replica_groups=rg,
                    ins=[ag_in_b.opt()], outs=[ag_out_b],
                )

            with tc.tile_pool(name="qkv", bufs=1) as qkv:
                qr_sb = qkv.tile([128, NH * TOK], F16, tag="qr")
                kr_sb = qkv.tile([128, NH * TOK], F16, tag="kr")
                v_sb = qkv.tile([128, (TOK // 128) * NH * 128], F16, tag="v")
                rC = qkv.tile([128, TOK], F16, tag="rC")
                rS = qkv.tile([128, TOK], F16, tag="rS")
                nc.sync.dma_start(rC[:], ropeC[:])
                nc.sync.dma_start(rS[:], ropeS[:])
                wq_sb = qkv.tile([128, HC * NH * 128], F16, tag="wq")
                wk_sb = qkv.tile([128, HC * NH * 128], F16, tag="wk")
                wv_sb = qkv.tile([128, HC * NH * 128], F16, tag="wv")
                nc.scalar.dma_start(wq_sb[:], wq[:])
                nc.scalar.dma_start(wk_sb[:], wk[:])
                nc.scalar.dma_start(wv_sb[:], wv[:])

                # ---------------- Stage B: qkv projections + RoPE ------------
                with (
                    tc.tile_pool(name="htc", bufs=30) as htc,
                    tc.tile_pool(name="qkpre", bufs=6) as qkpre,
                    tc.tile_pool(name="ropet", bufs=8) as ropet,
                    tc.tile_pool(name="psqk", bufs=4, space="PSUM") as psqk,
                    tc.tile_pool(name="psv", bufs=4, space="PSUM") as psv,
                ):
                    for tb in range(NCORES):
                        hts = []
                        for j in range(HC):
                            t = htc.tile([128, TPC], F16, tag="ht")
                            buf = ag_out_a if j < 8 else ag_out_b
                            jj = j % 8
                            nc.sync.dma_start(
                                t[:],
                                buf[tb * (HID // 2) + jj * 128:
                                    tb * (HID // 2) + (jj + 1) * 128, :],
                            )
                            hts.append(t)
                        for (w_sb, r_sb) in ((wq_sb, qr_sb), (wk_sb, kr_sb)):
                            for m in range(NH):
                                ps = psqk.tile([128, TPC], F32, tag="qk")
                                for j in range(HC):
                                    nc.tensor.matmul(
                                        ps[:],
                                        w_sb[:, j * (NH * 128) + m * 128:
                                             j * (NH * 128) + (m + 1) * 128],
                                        hts[j][:],
                                        start=(j == 0), stop=(j == HC - 1),
                                    )
                                pre = qkpre.tile([128, TPC], F16, tag="pre")
                                nc.scalar.activation(pre[:], ps[:], AF.Copy)
                                # RoPE: rows [0:64] even dims, [64:128] odd dims
                                col = m * TOK + tb * TPC
                                cs = slice(tb * TPC, (tb + 1) * TPC)
                                qe = pre[0:64, :]
                                qo = pre[64:128, :]
                                t1 = ropet.tile([64, TPC], F16, tag="t1")
                                t2 = ropet.tile([64, TPC], F16, tag="t2")
                                t3 = ropet.tile([64, TPC], F16, tag="t3")
                                t4 = ropet.tile([64, TPC], F16, tag="t4")
                                nc.vector.tensor_mul(t1[:], qe, rC[0:64, cs])
                                nc.vector.tensor_mul(t2[:], qo, rS[64:128, cs])
                                nc.vector.tensor_sub(
                                    r_sb[0:64, col:col + TPC], t1[:], t2[:])
                                nc.vector.tensor_mul(t3[:], qe, rS[0:64, cs])
                                nc.vector.tensor_mul(t4[:], qo, rC[64:128, cs])
                                nc.vector.tensor_add(
                                    r_sb[64:128, col:col + TPC], t3[:], t4[:])
                        for mt in range(4):
                            ps = psv.tile([128, NH * 128], F32, tag="v")
                            for j in range(HC):
                                nc.tensor.matmul(
                                    ps[:],
                                    hts[j][:, mt * 128:(mt + 1) * 128],
                                    wv_sb[:, j * (NH * 128):(j + 1) * (NH * 128)],
                                    start=(j == 0), stop=(j == HC - 1),
                                )
                            ti = tb * 4 + mt
                            nc.scalar.activation(
                                v_sb[:, ti * (NH * 128):(ti + 1) * (NH * 128)],
                                ps[:], AF.Copy)

                # ---------------- Stage C: attention -------------------------
                SB = S // TPC   # 4 query blocks per batch
                KCN = S // 128  # 16 key chunks per batch
                with (
                    tc.tile_pool(name="cp", bufs=5) as cp,
                    tc.tile_pool(name="pss", bufs=2, space="PSUM") as pss_p,
                    tc.tile_pool(name="pso", bufs=3, space="PSUM") as pso_p,
                    tc.tile_pool(name="psdn", bufs=1, space="PSUM") as psdn_p,
                ):
                    for m in range(NH):
                      for b in range(B):
                        if True:
                            qcol = m * TOK + b * S
                            for qb in range(SB):
                                pso = pso_p.tile([128, TPC], F32, tag="o")
                                den = cp.tile([128, TPC], F16, tag="den")
                                qsl = slice(qcol + qb * TPC, qcol + (qb + 1) * TPC)
                                for kg in range(KCN // 2):
                                    pss = pss_p.tile([128, 2 * TPC], F32, tag="s")
                                    for h_ in range(2):
                                        kc = kg * 2 + h_
                                        nc.tensor.matmul(
                                            pss[:, h_ * TPC:(h_ + 1) * TPC],
                                            kr_sb[:, qcol + kc * 128: qcol + (kc + 1) * 128],
                                            qr_sb[:, qsl],
                                            start=True, stop=True,
                                        )
                                    pt = cp.tile([128, 2 * TPC], F16, tag="pt")
                                    nc.scalar.activation(
                                        pt[:], pss[:], AF.Exp, scale=SCALE,
                                        bias=expb_b[:])
                                    if kg == 0:
                                        nc.vector.tensor_add(
                                            den[:], pt[:, 0:TPC], pt[:, TPC:2 * TPC])
                                    else:
                                        nc.vector.tensor_add(den[:], den[:], pt[:, 0:TPC])
                                        nc.vector.tensor_add(den[:], den[:], pt[:, TPC:2 * TPC])
                                    for h_ in range(2):
                                        kc = kg * 2 + h_
                                        ti = b * (S // 128) + kc
                                        nc.tensor.matmul(
                                            pso[:],
                                            v_sb[:, ti * (NH * 128) + m * 128:
                                                 ti * (NH * 128) + (m + 1) * 128],
                                            pt[:, h_ * TPC:(h_ + 1) * TPC],
                                            start=(kc == 0), stop=(kc == KCN - 1),
                                        )
                                psden = psdn_p.tile([1, TPC], F32, tag="dn")
                                nc.tensor.matmul(psden[:], ones_col16[:], den[:],
                                                 start=True, stop=True)
                                rec = cp.tile([1, TPC], F32, tag="rec")
                                nc.vector.reciprocal(rec[:], psden[:])
                                rb = cp.tile([128, TPC], F32, tag="rbs")
                                nc.gpsimd.partition_broadcast(rb[:], rec[:])
                                at = cp.tile([128, TPC], F16, tag="at")
                                nc.vector.tensor_mul(at[:], pso[:], rb[:])
                                row = (b * SB + qb) * 128
                                nc.sync.dma_start(
                                    a2a_in_m[m][row:row + 128, :], at[:])
                        if b == B - 1:
                            if single_core:
                                a2a_mid = dram.tile([NCORES * 128, TPC], F16,
                                                    name=f"a2am{m}")
                                nc.sync.dma_start(a2a_mid[:, :], a2a_in_m[m][:, :])
                                nc.sync.dma_start(a2a_out_m[m][:, :], a2a_mid[:, :])
                            else:
                                nc.gpsimd.collective_compute(
                                    "AllToAll", mybir.AluOpType.bypass,
                                    replica_groups=rg,
                                    ins=[a2a_in_m[m].opt()], outs=[a2a_out_m[m].opt()],
                                )

            with tc.tile_pool(name="late", bufs=1) as late:
                x2_sb = late.tile([128, HC * TPC], F32, tag="x2")
                h2_sb = late.tile([128, HC * TPC], F16, tag="h2")
                ff_sb = late.tile([128, FFC * TPC], F16, tag="ff")

                # ------------- Stage D: out_proj + residual + LN2 ------------
                with (
                    tc.tile_pool(name="atp", bufs=HC + 2) as atp,
                    tc.tile_pool(name="wop", bufs=4) as wop,
                    tc.tile_pool(name="lnD", bufs=4) as lnD,
                    tc.tile_pool(name="pso2", bufs=4, space="PSUM") as pso2_p,
                    tc.tile_pool(name="psstD", bufs=2, space="PSUM") as psstD,
                    tc.tile_pool(name="psbcD", bufs=2, space="PSUM") as psbcD,
                ):
                    ats = []
                    for j in range(HC):
                        t = atp.tile([128, TPC], F16, tag="at")
                        buf = a2a_out_m[j % 2]
                        r = j // 2
                        nc.sync.dma_start(t[:], buf[r * 128:(r + 1) * 128, :])
                        ats.append(t)
                    for mo in range(HC):
                        ws = wop.tile([128, HC * 128], F16, tag="wo")
                        nc.scalar.dma_start(ws[:], wo[mo * 128:(mo + 1) * 128, :])
                        ps = pso2_p.tile([128, TPC], F32, tag="o2")
                        for j in range(HC):
                            nc.tensor.matmul(
                                ps[:], ws[:, j * 128:(j + 1) * 128], ats[j][:],
                                start=(j == 0), stop=(j == HC - 1),
                            )
                        xt = lnD.tile([128, TPC], F32, tag="xres")
                        nc.sync.dma_start(xt[:], xT[mo * 128:(mo + 1) * 128, :])
                        nc.vector.tensor_add(
                            x2_sb[:, mo * TPC:(mo + 1) * TPC], ps[:], xt[:])

                    def get_x2(j):
                        return x2_sb[:, j * TPC:(j + 1) * TPC]

                    def put_h2(j, t2, gj, bj):
                        nc.gpsimd.tensor_scalar(
                            h2_sb[:, j * TPC:(j + 1) * TPC], t2[:], gj, bj,
                            MULT, ADD)

                    layernorm(get_x2, put_h2, g2_sb, b2_sb, lnD, psstD, psbcD)

                # ------------- Stage E: MLP ----------------------------------
                with (
                    tc.tile_pool(name="wf1p", bufs=4) as wf1p,
                    tc.tile_pool(name="wf2p", bufs=5) as wf2p,
                    tc.tile_pool(name="outp", bufs=3) as outp,
                    tc.tile_pool(name="psf1", bufs=4, space="PSUM") as psf1_p,
                    tc.tile_pool(name="psf2", bufs=4, space="PSUM") as psf2_p,
                ):
                    for mo in range(FFC):
                        ws = wf1p.tile([128, HC * 128], F16, tag="wf1")
                        nc.scalar.dma_start(ws[:], wf1[mo * 128:(mo + 1) * 128, :])
                        ps = psf1_p.tile([128, TPC], F32, tag="f1")
                        for j in range(HC):
                            nc.tensor.matmul(
                                ps[:], ws[:, j * 128:(j + 1) * 128],
                                h2_sb[:, j * TPC:(j + 1) * TPC],
                                start=(j == 0), stop=(j == HC - 1),
                            )
                        nc.scalar.activation(
                            ff_sb[:, mo * TPC:(mo + 1) * TPC], ps[:], AF.Gelu,
                            bias=zero_b[:])
                    for mo in range(HC):
                        ws = wf2p.tile([128, FFC * 128], F16, tag="wf2")
                        nc.scalar.dma_start(ws[:], wf2[mo * 128:(mo + 1) * 128, :])
                        ps = psf2_p.tile([128, TPC], F32, tag="f2")
                        for j in range(FFC):
                            nc.tensor.matmul(
                                ps[:], ws[:, j * 128:(j + 1) * 128],
                                ff_sb[:, j * TPC:(j + 1) * TPC],
                                start=(j == 0), stop=(j == FFC - 1),
                            )
                        ot = outp.tile([128, TPC], F32, tag="ot")
                        nc.vector.tensor_add(
                            ot[:], ps[:], x2_sb[:, mo * TPC:(mo + 1) * TPC])
                        nc.sync.dma_start(outT[mo * 128:(mo + 1) * 128, :], ot[:])
    return nc


def _build():
    if "nc" in _CACHE:
        return _CACHE["nc"]
    nc = bacc.Bacc(
        "TRN2", target_bir_lowering=False, debug=False,
        enable_asserts=True, num_devices=NCORES,
    )
    _emit(nc)
    nc.compile()
    _CACHE["nc"] = nc
    return nc


def _strips(wT, n_strips):
    # wT [K, n_strips*128] -> [n_strips*128, K] where strip m rows are
    # [128 partitions, K/128 chunks * 128] in SBUF lhsT layout
    K = wT.shape[0]
    kc = K // 128
    out = np.empty((n_strips * 128, K), dtype=np.float16)
    for m_ in range(n_strips):
        s = wT[:, m_ * 128:(m_ + 1) * 128]          # [K, 128]
        s = s.reshape(kc, 128, 128).transpose(1, 0, 2).reshape(128, K)
        out[m_ * 128:(m_ + 1) * 128, :] = s
    return out


def prepare_inputs(x, pe, w_qkv, w_out, w_fc1, w_fc2, g1, b1, g2, b2):
    x = np.asarray(x, np.float32)
    pe = np.asarray(pe, np.float32)
    w_qkv = np.asarray(w_qkv, np.float32)
    w_out = np.asarray(w_out, np.float32)
    w_fc1 = np.asarray(w_fc1, np.float32)
    w_fc2 = np.asarray(w_fc2, np.float32)

    xf = x.reshape(TOK, HID)
    perm = np.r_[np.arange(0, 128, 2), np.arange(1, 128, 2)]

    ropeC = np.tile(pe[:, 0::2].T, (2, B)).astype(np.float16)   # [128, TOK]
    ropeS = np.tile(pe[:, 1::2].T, (2, B)).astype(np.float16)

    gb = [np.asarray(v, np.float32).reshape(HC, 128).T.copy()
          for v in (g1, b1, g2, b2)]

    wo_h = _strips(w_out.T.astype(np.float16), HC)        # w_out.T: [feat, out]
    wf1_h = _strips(w_fc1.T.astype(np.float16), FFC)      # [hid, ffn]
    wf2_h = _strips(w_fc2.T.astype(np.float16), HC)       # [ffn, hid]

    in_maps = []
    for c in range(NCORES):
        heads = [NH * c + i for i in range(NH)]
        # q/k rows with per-head even/odd permutation; v natural
        qrows = np.concatenate([w_qkv[h * D + perm] for h in heads])      # [256, HID]
        krows = np.concatenate([w_qkv[HID + h * D + perm] for h in heads])
        vrows = np.concatenate([w_qkv[2 * HID + h * D: 2 * HID + (h + 1) * D]
                                for h in heads])

        def wlay(rows):
            # rows [NH*128, HID] -> lhsT sbuf layout [128, HC, NH*128]
            t = rows.T.astype(np.float16)                  # [HID, NH*128]
            t = t.reshape(HC, 128, NH * 128).transpose(1, 0, 2)
            return t.reshape(128, HC * NH * 128)

        xTc = np.ascontiguousarray(xf[c * TPC:(c + 1) * TPC].T)  # [HID, TPC]
        in_maps.append({
            "xT": xTc,
            "wq": wlay(qrows), "wk": wlay(krows), "wv": wlay(vrows),
            "wo": wo_h, "wf1": wf1_h, "wf2": wf2_h,
            "g1": gb[0], "b1": gb[1], "g2": gb[2], "b2": gb[3],
            "ropeC": ropeC, "ropeS": ropeS,
        })
    return in_maps


def run(in_maps, **kwargs):
    nc = _build()
    return bass_utils.run_bass_kernel_spmd(
        nc, in_maps, core_ids=list(range(NCORES)), **kwargs
    )


def kernel(x, pe, w_qkv, w_out, w_fc1, w_fc2, g1, b1, g2, b2):
    in_maps = prepare_inputs(x, pe, w_qkv, w_out, w_fc1, w_fc2, g1, b1, g2, b2)
    res = run(in_maps)
    fullT = np.concatenate([res.results[c]["outT"] for c in range(NCORES)], axis=1)
    return np.ascontiguousarray(fullT.T).reshape(B, S, HID).astype(np.float32)



# revision 17
# speedup vs baseline: 1.3972x; 1.3972x over previous
"""MiniTransformerLayer on 8 Trainium2 NeuronCores — fp8 DoubleRow edition.

Sharding (as baseline): core c owns tokens [512c, 512(c+1)) and heads
{2c, 2c+1}; 2 AllGathers (LN1 out, fp8) + 2 AllToAlls (attn out, fp8).

Numerics:
  - all attention-side matmuls in fp8e4 DoubleRow (2 K-chunks per
    instruction, 0.5 cyc/row): qkv, scores (64-partition DR with heads
    stacked in partitions 0:64/64:128), attn@V, softmax denominator
    (ones-DR), out_proj.
  - MLP fc1/fc2 as 3-term hi/lo fp8 DR: W·A = Wh·Ah + (Wl·Ah + Wh·Al),
    weights hi/lo prepared on host, activations hi/lo on device.
    Measured end-to-end rel err ~4e-3 (same as fp16 MLP).
  - LayerNorm stats via fp16 ones-matmuls on x kept resident in fp16;
    per-chunk scale/offset via rank-1/rank-2 coefficient matmuls
    (g,b folded), applied on DVE/Pool.
  - fixed power-of-2 scales: weights x1024 (host), q/k/v fp8 at 16x,
    probs at 1x, attn out at 64x; unscaled in Act copies / stt epilogues.
"""

import sys

sys.path.insert(0, "/opt/trn_rl_repo")

import numpy as np
import ml_dtypes

import concourse.bass as bass
import concourse.bacc as bacc
import concourse.tile as tile
import concourse.mybir as mybir
from concourse import bass_utils

F8 = mybir.dt.float8e4
F16 = mybir.dt.float16
F32 = mybir.dt.float32
AF = mybir.ActivationFunctionType
DR = mybir.MatmulPerfMode.DoubleRow
E4 = ml_dtypes.float8_e4m3

NCORES = 8
B, S, HID, HEADS, D, FFN = 2, 2048, 2048, 16, 128, 4096
TOK = B * S            # 4096 flat tokens
TPC = TOK // NCORES    # 512 tokens per core
HC = HID // 128        # 16 hidden chunks
FFC = FFN // 128       # 32 ffn chunks
NH = HEADS // NCORES   # 2 heads per core
SCALE = 1.0 / float(np.sqrt(D))
EXP_BIAS = -3.0
EPS = 1e-5
WS = 1024.0            # host weight scale
QS = 2.0 ** -6         # psum(1024 q) -> 16 q
MULT, ADD, SUB = (mybir.AluOpType.mult, mybir.AluOpType.add,
                  mybir.AluOpType.subtract)

_CACHE = {}


def _emit(nc, single_core=False):
    xT = nc.dram_tensor("xT", [HID, TPC], F16, kind="ExternalInput")
    wq = nc.dram_tensor("wq", [128, HC * 2 * 128], F8, kind="ExternalInput")
    wk = nc.dram_tensor("wk", [128, HC * 2 * 128], F8, kind="ExternalInput")
    wv = nc.dram_tensor("wv", [128, HC * 256], F8, kind="ExternalInput")
    wo = nc.dram_tensor("wo", [HC * 128, HC * 128], F8, kind="ExternalInput")
    wf1 = nc.dram_tensor("wf1", [FFC * 128, HC * 2 * 128], F8,
                         kind="ExternalInput")
    wf2 = nc.dram_tensor("wf2", [HC * 128, FFC * 2 * 128], F8,
                         kind="ExternalInput")
    g1b1 = nc.dram_tensor("g1b1", [2, HID], F16, kind="ExternalInput")
    g2b2 = nc.dram_tensor("g2b2", [2, HID], F16, kind="ExternalInput")
    ropeC = nc.dram_tensor("ropeC", [128, TOK], F16, kind="ExternalInput")
    ropeS = nc.dram_tensor("ropeS", [128, TOK], F16, kind="ExternalInput")
    outT = nc.dram_tensor("outT", [HID, TPC], F32, kind="ExternalOutput")

    rg = [list(range(NCORES))]

    with tile.TileContext(nc) as tc:
        with (
            nc.allow_low_precision(reason="fp8 kernel: quantized by design"),
            tc.tile_pool(name="const", bufs=1) as const,
            tc.tile_pool(name="dram", bufs=1, space="DRAM") as dram,
        ):
            ones16 = const.tile([128, 1], F16, tag="on16")
            nc.vector.memset(ones16[:], 1.0)
            onesr16 = const.tile([1, 128], F16, tag="onr16")
            nc.vector.memset(onesr16[:], 1.0)
            ones8q = const.tile([128, 2, 32], F8, tag="on8q")
            nc.vector.memset(ones8q[:], 0.25)
            eps_b = const.tile([1, 1], F32, tag="epsb")
            nc.vector.memset(eps_b[:], EPS)
            zero1_b = const.tile([1, 1], F32, tag="z1b")
            nc.vector.memset(zero1_b[:], 0.0)
            expb_b = const.tile([128, 1], F32, tag="expb")
            nc.vector.memset(expb_b[:], EXP_BIAS)
            g1b1_sb = const.tile([2, HID], F16, tag="g1b1")
            g2b2_sb = const.tile([2, HID], F16, tag="g2b2")
            nc.scalar.dma_start(g1b1_sb[:], g1b1[:])
            nc.scalar.dma_start(g2b2_sb[:], g2b2[:])

            # resident activations
            x16 = const.tile([128, HC, TPC], F16, tag="x16")
            q_sb = const.tile([128, 2, TOK], F8, tag="qsb")
            k_sb = const.tile([128, 2, TOK], F8, tag="ksb")
            v_sb = const.tile([128, TOK // 128, 256], F8, tag="vsb")
            at_all = const.tile([128, HC, TPC], F8, tag="atall")
            x2_sb = const.tile([128, HC, TPC], F16, tag="x2")
            h2x = const.tile([128, HC, 2, TPC], F8, tag="h2x")
            ffx = const.tile([128, FFC, 2, TPC], F8, tag="ffx")

            ag_in_a = dram.tile([HID // 2, TPC], F8)
            ag_in_b = dram.tile([HID // 2, TPC], F8)
            a2a_in_m = [dram.tile([NCORES * 128, TPC], F8, name=f"a2ai{m}")
                        for m in range(NH)]
            a2a_out_m = [dram.tile([NCORES * 128, TPC], F8, name=f"a2ao{m}")
                         for m in range(NH)]
            if single_core:
                ag_out_a = dram.tile([NCORES * HID // 2, TPC], F8)
                ag_out_b = dram.tile([NCORES * HID // 2, TPC], F8)
            else:
                ag_out_a = nc.dram_tensor(
                    "ag_out_a_sh", [NCORES * HID // 2, TPC], F8,
                    addr_space="Shared").ap()
                ag_out_b = nc.dram_tensor(
                    "ag_out_b_sh", [NCORES * HID // 2, TPC], F8,
                    addr_space="Shared").ap()

            def layernorm(src, gb, put, lnp, psst, psbc):
                """src(j) -> [128, TPC] f16 AP; put(j, c1_psum, c0_psum)."""
                ps_sx = psst.tile([1, TPC], F32, tag="st")
                ps_sq = psst.tile([1, TPC], F32, tag="st")
                for j in range(HC):
                    s = src(j)
                    sqt = lnp.tile([128, TPC], F16, tag="sqt")
                    nc.vector.tensor_mul(sqt[:], s, s)
                    nc.tensor.matmul(ps_sx[:], ones16[:], s,
                                     start=(j == 0), stop=(j == HC - 1))
                    nc.tensor.matmul(ps_sq[:], ones16[:], sqt[:],
                                     start=(j == 0), stop=(j == HC - 1))
                return _ln_coeff(ps_sx, ps_sq, gb, put, lnp, psbc)

            def _ln_coeff(ps_sx, ps_sq, gb, put, lnp, psbc):
                mu = lnp.tile([1, TPC], F32, tag="mu", bufs=1)
                m2 = lnp.tile([1, TPC], F32, tag="m2", bufs=1)
                var = lnp.tile([1, TPC], F32, tag="var", bufs=1)
                lnv = lnp.tile([1, TPC], F32, tag="lnv", bufs=1)
                rstd16 = lnp.tile([1, TPC], F16, tag="rstd", bufs=1)
                mrs_ones = lnp.tile([2, TPC], F16, tag="mrso", bufs=1)
                nc.vector.tensor_scalar_mul(mu[:], ps_sx[:], 1.0 / HID)
                nc.vector.tensor_scalar_mul(m2[:], ps_sq[:], 1.0 / HID)
                nc.vector.tensor_mul(var[:], mu[:], mu[:])
                nc.vector.tensor_sub(var[:], m2[:], var[:])
                nc.scalar.activation(lnv[:], var[:], AF.Ln, bias=eps_b[:])
                nc.scalar.activation(rstd16[:], lnv[:], AF.Exp,
                                     bias=zero1_b[:], scale=-0.5)
                nc.vector.memset(mrs_ones[:], 1.0)
                nc.vector.tensor_mul(mrs_ones[0:1, :], mu[:], rstd16[:])
                nc.vector.tensor_scalar_mul(mrs_ones[0:1, :],
                                            mrs_ones[0:1, :], -1.0)
                for j in range(HC):
                    cs = slice(j * 128, (j + 1) * 128)
                    ps_c1 = psbc.tile([128, TPC], F32, tag="bc")
                    ps_c0 = psbc.tile([128, TPC], F32, tag="bc")
                    nc.tensor.matmul(ps_c1[:], gb[0:1, cs], rstd16[:],
                                     start=True, stop=True)
                    nc.tensor.matmul(ps_c0[:], gb[0:2, cs], mrs_ones[:],
                                     start=True, stop=True)
                    put(j, ps_c1, ps_c0)

            # ---------------- Stage A: load x, LN1, AllGather ----------------
            with (
                tc.tile_pool(name="lnA", bufs=4) as lnA,
                tc.tile_pool(name="psstA", bufs=2, space="PSUM") as psstA,
                tc.tile_pool(name="psbcA", bufs=4, space="PSUM") as psbcA,
            ):
                for j in range(HC):
                    nc.sync.dma_start(x16[:, j, :],
                                      xT[j * 128:(j + 1) * 128, :])

                def get_x(j):
                    return x16[:, j, :]

                def put_h1(j, ps_c1, ps_c0):
                    # GPSIMD can't read PSUM: DVE does the psum mul, Act
                    # copies c0 to SBUF, Pool does the SBUF-only add.
                    t1 = lnA.tile([128, TPC], F16, tag="t1")
                    c0s = lnA.tile([128, TPC], F16, tag="c0s")
                    h1 = lnA.tile([128, TPC], F8, tag="h1")
                    nc.vector.tensor_mul(t1[:], x16[:, j, :], ps_c1[:])
                    nc.scalar.activation(c0s[:], ps_c0[:], AF.Copy)
                    nc.gpsimd.tensor_add(h1[:], t1[:], c0s[:])
                    half, jj = (ag_in_a, j) if j < 8 else (ag_in_b, j - 8)
                    nc.sync.dma_start(half[jj * 128:(jj + 1) * 128, :],
                                      h1[:])

                layernorm(get_x, g1b1_sb, put_h1, lnA, psstA, psbcA)

            H2 = HID // 2
            if single_core:
                # AllGather stand-in (honest cost: quarter-width write per
                # rank slot, same structure as baseline at fp8 width)
                for r in range(NCORES):
                    nc.sync.dma_start(ag_out_a[r * H2:(r + 1) * H2, 0:TPC // 4],
                                      ag_in_a[:, 0:TPC // 4])
                    nc.sync.dma_start(ag_out_b[r * H2:(r + 1) * H2, 0:TPC // 4],
                                      ag_in_b[:, 0:TPC // 4])
            else:
                nc.gpsimd.collective_compute(
                    "AllGather", mybir.AluOpType.bypass, replica_groups=rg,
                    ins=[ag_in_a.opt()], outs=[ag_out_a])
                nc.gpsimd.collective_compute(
                    "AllGather", mybir.AluOpType.bypass, replica_groups=rg,
                    ins=[ag_in_b.opt()], outs=[ag_out_b])

            # ---------------- Stage B: qkv + RoPE ----------------------------
            with (
                tc.tile_pool(name="bwt", bufs=1) as bwt,
                tc.tile_pool(name="htc", bufs=18) as htc,
                tc.tile_pool(name="qkpre", bufs=4) as qkpre,
                tc.tile_pool(name="ropet", bufs=8) as ropet,
                tc.tile_pool(name="psqk", bufs=2, space="PSUM") as psqk,
                tc.tile_pool(name="psv", bufs=2, space="PSUM") as psv,
            ):
                rC = bwt.tile([128, TOK], F16, tag="rC")
                rS = bwt.tile([128, TOK], F16, tag="rS")
                nc.scalar.dma_start(rC[:], ropeC[:])
                nc.scalar.dma_start(rS[:], ropeS[:])
                wq_sb = bwt.tile([128, HC, 2, 128], F8, tag="wq")
                wk_sb = bwt.tile([128, HC, 2, 128], F8, tag="wk")
                wv_sb = bwt.tile([128, HC, 256], F8, tag="wv")
                nc.scalar.dma_start(wq_sb[:].opt(), wq[:])
                nc.scalar.dma_start(wk_sb[:].opt(), wk[:])
                nc.scalar.dma_start(wv_sb[:].opt(), wv[:])
                for tb in range(NCORES):
                    hts = []
                    for jp in range(8):
                        t = htc.tile([128, 2, TPC], F8, tag="ht")
                        for h_ in range(2):
                            j = 2 * jp + h_
                            buf = ag_out_a if j < 8 else ag_out_b
                            jj = j % 8
                            nc.sync.dma_start(
                                t[:, h_, :],
                                buf[tb * H2 + jj * 128:
                                    tb * H2 + (jj + 1) * 128, :])
                        hts.append(t[:])
                    tcols = slice(tb * TPC, (tb + 1) * TPC)
                    # q and k: two chains (even/odd dims), heads stacked
                    for (w_sb, dst) in ((wq_sb, q_sb), (wk_sb, k_sb)):
                        pre = qkpre.tile([128, 2, TPC], F16, tag="pre")
                        for ch in range(2):
                            ps = psqk.tile([128, TPC], F32, tag="qk")
                            for jp in range(8):
                                nc.tensor.matmul(
                                    ps[:], w_sb[:, 2 * jp:2 * jp + 2, ch, :],
                                    hts[jp], start=(jp == 0), stop=(jp == 7),
                                    perf_mode=DR)
                            nc.scalar.activation(pre[:, ch, :], ps[:],
                                                 AF.Copy, scale=QS)
                        # RoPE (q on DVE, k on Pool)
                        eng = nc.vector if dst is q_sb else nc.gpsimd
                        t1 = ropet.tile([128, TPC], F16, tag="t1")
                        t2 = ropet.tile([128, TPC], F16, tag="t2")
                        t3 = ropet.tile([128, TPC], F16, tag="t3")
                        t4 = ropet.tile([128, TPC], F16, tag="t4")
                        eng.tensor_mul(t1[:], pre[:, 0, :], rC[:, tcols])
                        eng.tensor_mul(t2[:], pre[:, 1, :], rS[:, tcols])
                        eng.tensor_mul(t3[:], pre[:, 0, :], rS[:, tcols])
                        eng.tensor_mul(t4[:], pre[:, 1, :], rC[:, tcols])
                        eng.tensor_sub(dst[:, 0, tcols], t1[:], t2[:])
                        eng.tensor_add(dst[:, 1, tcols], t3[:], t4[:])
                    # v: token-major, 4 sub-chunks of 128 tokens
                    for mt in range(4):
                        ps = psv.tile([128, 256], F32, tag="v")
                        for jp in range(8):
                            nc.tensor.matmul(
                                ps[:], hts[jp][:, :, mt * 128:(mt + 1) * 128],
                                wv_sb[:, 2 * jp:2 * jp + 2, :],
                                start=(jp == 0), stop=(jp == 7), perf_mode=DR)
                        nc.scalar.activation(v_sb[:, tb * 4 + mt, :], ps[:],
                                             AF.Copy, scale=QS)

            # ---------------- Stage C: attention ------------------------------
            SB = S // TPC   # 4 query blocks per batch
            KCN = S // 128  # 16 key chunks per batch
            with (
                tc.tile_pool(name="cp", bufs=4) as cp,
                tc.tile_pool(name="ptp", bufs=3) as ptp,
                tc.tile_pool(name="pss", bufs=2, space="PSUM") as pss_p,
                tc.tile_pool(name="pso", bufs=2, space="PSUM") as pso_p,
                tc.tile_pool(name="psdn", bufs=2, space="PSUM") as psdn_p,
            ):
                for m in range(NH):
                    mr = slice(64 * m, 64 * (m + 1))
                    for b in range(B):
                        for qb in range(SB):
                            qsl = slice(b * S + qb * TPC, b * S + (qb + 1) * TPC)
                            pso = pso_p.tile([128, TPC], F32, tag="o")
                            psden = psdn_p.tile([32, TPC], F32, tag="dn")
                            for kg in range(KCN // 2):
                                pss = pss_p.tile([128, 2, TPC], F32, tag="s")
                                for h_ in range(2):
                                    kc = kg * 2 + h_
                                    ksl = slice(b * S + kc * 128,
                                                b * S + (kc + 1) * 128)
                                    nc.tensor.matmul(
                                        pss[:, h_, :], k_sb[mr, :, ksl],
                                        q_sb[mr, :, qsl],
                                        start=True, stop=True, perf_mode=DR)
                                pt = ptp.tile([128, 2, TPC], F8, tag="pt")
                                nc.scalar.activation(
                                    pt[:].opt(), pss[:].opt(), AF.Exp,
                                    scale=SCALE / 256.0, bias=expb_b[:])
                                nc.tensor.matmul(
                                    psden[:], ones8q[:], pt[:],
                                    start=(kg == 0), stop=(kg == KCN // 2 - 1),
                                    perf_mode=DR)
                                nc.tensor.matmul(
                                    pso[:],
                                    v_sb[:, b * 16 + 2 * kg:b * 16 + 2 * kg + 2,
                                         m * 128:(m + 1) * 128],
                                    pt[:],
                                    start=(kg == 0), stop=(kg == KCN // 2 - 1),
                                    perf_mode=DR)
                            rec16 = cp.tile([1, TPC], F16, tag="rec")
                            nc.vector.reciprocal(rec16[:], psden[0:1, :])
                            rb = cp.tile([128, TPC], F16, tag="rb")
                            nc.gpsimd.partition_broadcast(rb[:], rec16[:])
                            at = cp.tile([128, TPC], F8, tag="at")
                            nc.vector.tensor_mul(at[:], pso[:], rb[:])
                            row = (b * SB + qb) * 128
                            nc.sync.dma_start(
                                a2a_in_m[m][row:row + 128, :], at[:])
                    if single_core:
                        a2a_mid = dram.tile([NCORES * 128, TPC], F8,
                                            name=f"a2am{m}")
                        nc.sync.dma_start(a2a_mid[:, :], a2a_in_m[m][:, :])
                        nc.sync.dma_start(a2a_out_m[m][:, :], a2a_mid[:, :])
                    else:
                        nc.gpsimd.collective_compute(
                            "AllToAll", mybir.AluOpType.bypass,
                            replica_groups=rg,
                            ins=[a2a_in_m[m].opt()], outs=[a2a_out_m[m].opt()])

            # ------------- Stage D: out_proj + residual + LN2 -----------------
            with (
                tc.tile_pool(name="wop", bufs=4) as wop,
                tc.tile_pool(name="lnD", bufs=4) as lnD,
                tc.tile_pool(name="pso2", bufs=3, space="PSUM") as pso2_p,
                tc.tile_pool(name="psstD", bufs=2, space="PSUM") as psstD,
                tc.tile_pool(name="psbcD", bufs=3, space="PSUM") as psbcD,
            ):
                # at_all j-order: [even heads 0..7 | odd heads 8..15] so that
                # DR pairs stay within one AllToAll's output
                for jj in range(HC):
                    mm, r = (0, jj) if jj < 8 else (1, jj - 8)
                    nc.sync.dma_start(at_all[:, jj, :],
                                      a2a_out_m[mm][r * 128:(r + 1) * 128, :])
                ps_sx2 = psstD.tile([1, TPC], F32, tag="st")
                ps_sq2 = psstD.tile([1, TPC], F32, tag="st")
                for mo in range(HC):
                    ws = wop.tile([128, HC, 128], F8, tag="wo")
                    nc.scalar.dma_start(ws[:].opt(),
                                        wo[mo * 128:(mo + 1) * 128, :])
                    ps = pso2_p.tile([128, TPC], F32, tag="o2")
                    for jp in range(8):
                        nc.tensor.matmul(
                            ps[:], ws[:, 2 * jp:2 * jp + 2, :],
                            at_all[:, 2 * jp:2 * jp + 2, :],
                            start=(jp == 0), stop=(jp == 7), perf_mode=DR)
                    # x2 = psum * 2^-16 + x  (wo 1024x, at 64x)
                    nc.vector.scalar_tensor_tensor(
                        x2_sb[:, mo, :], ps[:], 2.0 ** -16, x16[:, mo, :],
                        MULT, ADD)
                    sq2 = lnD.tile([128, TPC], F16, tag="sq2")
                    nc.gpsimd.tensor_mul(sq2[:], x2_sb[:, mo, :],
                                         x2_sb[:, mo, :])
                    nc.tensor.matmul(ps_sx2[:], ones16[:], x2_sb[:, mo, :],
                                     start=(mo == 0), stop=(mo == HC - 1))
                    nc.tensor.matmul(ps_sq2[:], ones16[:], sq2[:],
                                     start=(mo == 0), stop=(mo == HC - 1))

                def put_h2(j, ps_c1, ps_c0):
                    t1 = lnD.tile([128, TPC], F16, tag="t1")
                    c0s = lnD.tile([128, TPC], F16, tag="c0s")
                    h2f = lnD.tile([128, TPC], F16, tag="h2f")
                    nc.vector.tensor_mul(t1[:], x2_sb[:, j, :], ps_c1[:])
                    nc.scalar.activation(c0s[:], ps_c0[:], AF.Copy)
                    nc.gpsimd.tensor_add(h2f[:], t1[:], c0s[:])
                    nc.scalar.activation(h2x[:, j, 0, :], h2f[:], AF.Copy)
                    nc.gpsimd.tensor_sub(h2x[:, j, 1, :], h2f[:],
                                         h2x[:, j, 0, :])

                _ln_coeff(ps_sx2, ps_sq2, g2b2_sb, put_h2, lnD, psbcD)

            # ------------- Stage E: MLP ---------------------------------------
            with (
                tc.tile_pool(name="wf1p", bufs=4) as wf1p,
                tc.tile_pool(name="wf2p", bufs=3) as wf2p,
                tc.tile_pool(name="mlt", bufs=4) as mlt,
                tc.tile_pool(name="psf1", bufs=3, space="PSUM") as psf1_p,
                tc.tile_pool(name="psf2", bufs=3, space="PSUM") as psf2_p,
            ):
                for mo in range(FFC):
                    ws = wf1p.tile([128, HC, 2, 128], F8, tag="wf1")
                    nc.scalar.dma_start(ws[:].opt(),
                                        wf1[mo * 128:(mo + 1) * 128, :])
                    ps = psf1_p.tile([128, TPC], F32, tag="f1")
                    for jp in range(8):  # hi*hi
                        nc.tensor.matmul(
                            ps[:], ws[:, 2 * jp:2 * jp + 2, 1, :],
                            h2x[:, 2 * jp:2 * jp + 2, 0, :],
                            start=(jp == 0), stop=False, perf_mode=DR)
                    for j in range(HC):  # cross: Wl*Ah + Wh*Al
                        nc.tensor.matmul(
                            ps[:], ws[:, j, :, :], h2x[:, j, :, :],
                            start=False, stop=(j == HC - 1), perf_mode=DR)
                    ff16 = mlt.tile([128, TPC], F16, tag="ff16")
                    nc.scalar.activation(ff16[:], ps[:], AF.Gelu,
                                         scale=1.0 / WS)
                    eng, oeng = ((nc.gpsimd, nc.vector) if mo % 2 == 0
                                 else (nc.vector, nc.gpsimd))
                    eng.tensor_copy(ffx[:, mo, 0, :], ff16[:])
                    oeng.tensor_sub(ffx[:, mo, 1, :], ff16[:],
                                    ffx[:, mo, 0, :])
                for mo in range(HC):
                    ws = wf2p.tile([128, FFC, 2, 128], F8, tag="wf2")
                    nc.scalar.dma_start(ws[:].opt(),
                                        wf2[mo * 128:(mo + 1) * 128, :])
                    ps = psf2_p.tile([128, TPC], F32, tag="f2")
                    for jp in range(FFC // 2):  # hi*hi
                        nc.tensor.matmul(
                            ps[:], ws[:, 2 * jp:2 * jp + 2, 1, :],
                            ffx[:, 2 * jp:2 * jp + 2, 0, :],
                            start=(jp == 0), stop=False, perf_mode=DR)
                    for j in range(FFC):  # cross
                        nc.tensor.matmul(
                            ps[:], ws[:, j, :, :], ffx[:, j, :, :],
                            start=False, stop=(j == FFC - 1), perf_mode=DR)
                    ot = mlt.tile([128, TPC], F32, tag="ot")
                    nc.vector.scalar_tensor_tensor(
                        ot[:], ps[:], 1.0 / WS, x2_sb[:, mo, :], MULT, ADD)
                    nc.sync.dma_start(outT[mo * 128:(mo + 1) * 128, :], ot[:])
    return nc


def _build():
    if "nc" in _CACHE:
        return _CACHE["nc"]
    nc = bacc.Bacc(
        "TRN2", target_bir_lowering=False, debug=False,
        enable_asserts=True, num_devices=NCORES,
    )
    _emit(nc)
    nc.compile()
    _CACHE["nc"] = nc
    return nc


def _q8(v):
    return np.asarray(v, np.float32).astype(E4)


def prepare_inputs(x, pe, w_qkv, w_out, w_fc1, w_fc2, g1, b1, g2, b2):
    x = np.asarray(x, np.float32)
    pe = np.asarray(pe, np.float32)
    w_qkv = np.asarray(w_qkv, np.float32)
    w_out = np.asarray(w_out, np.float32)
    w_fc1 = np.asarray(w_fc1, np.float32)
    w_fc2 = np.asarray(w_fc2, np.float32)

    xf = x.reshape(TOK, HID)
    ropeC = np.tile(pe[:, 0::2].T, (2, B)).astype(np.float16)   # [128, TOK]
    ropeS = np.tile(pe[:, 1::2].T, (2, B)).astype(np.float16)
    g1b1 = np.stack([np.asarray(g1, np.float32),
                     np.asarray(b1, np.float32)]).astype(np.float16)
    g2b2 = np.stack([np.asarray(g2, np.float32),
                     np.asarray(b2, np.float32)]).astype(np.float16)

    # out_proj: strips over reordered contraction chunks
    # jj<8 -> head 2*jj (from a2a[0]); jj>=8 -> head 2*(jj-8)+1 (a2a[1])
    head_of = [2 * jj if jj < 8 else 2 * (jj - 8) + 1 for jj in range(HC)]
    wo_h = np.empty((HC * 128, HC * 128), dtype=E4)
    wt = (w_out * WS).astype(np.float32)   # [out, feat]
    for mo in range(HC):
        blk = np.empty((128, HC, 128), np.float32)
        for jj in range(HC):
            h = head_of[jj]
            # [p, col] = wt[mo*128+col, h*128+p]
            blk[:, jj, :] = wt[mo * 128:(mo + 1) * 128,
                               h * 128:(h + 1) * 128].T
        wo_h[mo * 128:(mo + 1) * 128] = _q8(blk.reshape(128, HC * 128))

    def hilo_strips(w, n_strips, kc):
        # w [n_strips*128, kc*128] (x WS): strip layout [128, kc, 2, 128]
        w = (w * WS).astype(np.float32)
        hi = _q8(w).astype(np.float32)
        lo = _q8(w - hi).astype(np.float32)
        out = np.empty((n_strips * 128, kc * 2 * 128), dtype=E4)
        for mo in range(n_strips):
            rows = slice(mo * 128, (mo + 1) * 128)
            # [p, j, s, col]: s=0 lo, s=1 hi; value w[mo*128+col, j*128+p]
            blk = np.empty((128, kc, 2, 128), np.float32)
            wl = lo[rows]; wh = hi[rows]    # [128(col), kc*128]
            blk[:, :, 0, :] = wl.reshape(128, kc, 128).transpose(2, 1, 0)
            blk[:, :, 1, :] = wh.reshape(128, kc, 128).transpose(2, 1, 0)
            out[rows] = _q8(blk.reshape(128, kc * 2 * 128))
        return out

    wf1_h = hilo_strips(w_fc1, FFC, HC)     # [4096, HC*2*128]
    wf2_h = hilo_strips(w_fc2, HC, FFC)     # [2048, FFC*2*128]

    in_maps = []
    for c in range(NCORES):
        hsl = slice(2 * c * D, (2 * c + 2) * D)

        def qk_lay(rows):
            # rows [256, HID] (2 heads) -> [128(p), HC(j), 2(ch), 128(m,pp)]
            r = (rows * WS).astype(np.float32)
            t = r.reshape(2, 64, 2, HC, 128)       # [m, pp, ch, j, p]
            t = t.transpose(4, 3, 2, 0, 1)          # [p, j, ch, m, pp]
            return _q8(t.reshape(128, HC * 2 * 128))

        qrows = w_qkv[hsl]
        krows = w_qkv[HID + 2 * c * D: HID + (2 * c + 2) * D]
        vrows = w_qkv[2 * HID + 2 * c * D: 2 * HID + (2 * c + 2) * D]
        vv = (vrows * WS).astype(np.float32).reshape(2, 128, HC, 128)
        wv_c = _q8(vv.transpose(3, 2, 0, 1).reshape(128, HC * 256))

        xTc = np.ascontiguousarray(
            xf[c * TPC:(c + 1) * TPC].T).astype(np.float16)
        in_maps.append({
            "xT": xTc,
            "wq": qk_lay(qrows), "wk": qk_lay(krows), "wv": wv_c,
            "wo": wo_h, "wf1": wf1_h, "wf2": wf2_h,
            "g1b1": g1b1, "g2b2": g2b2,
            "ropeC": ropeC, "ropeS": ropeS,
        })
    return in_maps


def run(in_maps, **kwargs):
    nc = _build()
    return bass_utils.run_bass_kernel_spmd(
        nc, in_maps, core_ids=list(range(NCORES)), **kwargs
    )


def kernel(x, pe, w_qkv, w_out, w_fc1, w_fc2, g1, b1, g2, b2):
    in_maps = prepare_inputs(x, pe, w_qkv, w_out, w_fc1, w_fc2, g1, b1, g2, b2)
    res = run(in_maps)
    fullT = np.concatenate([res.results[c]["outT"] for c in range(NCORES)],
                           axis=1)
    return np.ascontiguousarray(fullT.T).reshape(B, S, HID).astype(np.float32)


# revision 25
# speedup vs baseline: 1.4581x; 1.0436x over previous
"""MiniTransformerLayer on 8 Trainium2 NeuronCores — fp8 DoubleRow edition.

Sharding (as baseline): core c owns tokens [512c, 512(c+1)) and heads
{2c, 2c+1}; 2 AllGathers (LN1 out, fp8) + 2 AllToAlls (attn out, fp8).

Numerics:
  - all attention-side matmuls in fp8e4 DoubleRow (2 K-chunks per
    instruction, 0.5 cyc/row): qkv, scores (64-partition DR with heads
    stacked in partitions 0:64/64:128), attn@V, softmax denominator
    (ones-DR), out_proj.
  - MLP fc1/fc2 as 3-term hi/lo fp8 DR: W*A = Wh*Ah + (Wl*Ah + Wh*Al),
    weights hi/lo prepared on host, activations hi/lo on device.
  - softmax exp split across engines: Act native exp, plus a
    Schraudolph exp2 bit-trick pipeline (DVE f32->int32 convert from
    PSUM, Pool bitcast copy to fp8). The denominator cancels the shared
    approximation bias; measured end-to-end rel err ~4e-3.
  - LayerNorm stats via fp16 ones-matmuls; per-chunk scale/offset via
    rank-1/rank-2 coefficient matmuls (g,b folded).
  - fixed power-of-2 scales: weights x1024 (host), q/k/v fp8 at 16x,
    probs at 1x, attn out at 64x; unscaled in Act copies / stt epilogues.

Schedule notes: every dma_start costs ~630ns of serialized HWDGE
descriptor-generation, so transfers are batched through 3D DRAM
tensors + transposed access patterns (gathered h: 2 DMAs per token
block; attention outputs: 1 DMA per (head, batch); weights: 2 strips
per DMA). qkv for batch-1 token blocks is woven into the (head0,
batch0) attention blocks; LN stats matmuls lag their elementwise
producers to stay off the critical path.
"""

import sys

sys.path.insert(0, "/opt/trn_rl_repo")

import numpy as np
import ml_dtypes

import concourse.bass as bass
import concourse.bacc as bacc
import concourse.tile as tile
import concourse.mybir as mybir
from concourse import bass_utils

F8 = mybir.dt.float8e4
F16 = mybir.dt.float16
F32 = mybir.dt.float32
I32 = mybir.dt.int32
AF = mybir.ActivationFunctionType
DR = mybir.MatmulPerfMode.DoubleRow
E4 = ml_dtypes.float8_e4m3

NCORES = 8
B, S, HID, HEADS, D, FFN = 2, 2048, 2048, 16, 128, 4096
TOK = B * S            # 4096 flat tokens
TPC = TOK // NCORES    # 512 tokens per core
HC = HID // 128        # 16 hidden chunks
FFC = FFN // 128       # 32 ffn chunks
NH = HEADS // NCORES   # 2 heads per core
SCALE = 1.0 / float(np.sqrt(D))
EXP_BIAS = -3.0
EPS = 1e-5
WS = 1024.0            # host weight scale
QS = 2.0 ** -6         # psum(1024 q) -> 16 q
MULT, ADD, SUB = (mybir.AluOpType.mult, mybir.AluOpType.add,
                  mybir.AluOpType.subtract)
# Schraudolph exp2 bit trick: int32 bits = trunc(z*A + BC) ~ exp(z) bits
EXP_A = 12102203.161561485          # 2^23 / ln2
EXP_BC = 1064866805.0               # 127*2^23 - 366393
FE_SCALE = EXP_A * SCALE / 256.0
FE_BIAS = EXP_BC + EXP_A * EXP_BIAS

_CACHE = {}


def _emit(nc, single_core=False):
    xT = nc.dram_tensor("xT", [HC, 128, TPC], F16, kind="ExternalInput")
    wq = nc.dram_tensor("wq", [128, HC * 2 * 128], F8, kind="ExternalInput")
    wk = nc.dram_tensor("wk", [128, HC * 2 * 128], F8, kind="ExternalInput")
    wv = nc.dram_tensor("wv", [128, HC * 256], F8, kind="ExternalInput")
    wo = nc.dram_tensor("wo", [HC, 128, HC * 128], F8, kind="ExternalInput")
    wf1 = nc.dram_tensor("wf1", [FFC, 128, HC * 2 * 128], F8,
                         kind="ExternalInput")
    wf2 = nc.dram_tensor("wf2", [HC, 128, FFC * 2 * 128], F8,
                         kind="ExternalInput")
    g1b1 = nc.dram_tensor("g1b1", [2, HID], F16, kind="ExternalInput")
    g2b2 = nc.dram_tensor("g2b2", [2, HID], F16, kind="ExternalInput")
    ropeC = nc.dram_tensor("ropeC", [128, TOK], F16, kind="ExternalInput")
    ropeS = nc.dram_tensor("ropeS", [128, TOK], F16, kind="ExternalInput")
    outT = nc.dram_tensor("outT", [HC, 128, TPC], F32, kind="ExternalOutput")

    rg = [list(range(NCORES))]

    with tile.TileContext(nc) as tc:
        with (
            nc.allow_low_precision(reason="fp8 kernel: quantized by design"),
            tc.tile_pool(name="const", bufs=1) as const,
            tc.tile_pool(name="dram", bufs=1, space="DRAM") as dram,
            tc.tile_pool(name="wop", bufs=2) as wop,
        ):
            ones16 = const.tile([128, 1], F16, tag="on16")
            nc.vector.memset(ones16[:], 1.0)
            ones8q = const.tile([128, 2, 32], F8, tag="on8q")
            nc.vector.memset(ones8q[:], 0.25)
            eps_b = const.tile([1, 1], F32, tag="epsb")
            nc.vector.memset(eps_b[:], EPS)
            zero1_b = const.tile([1, 1], F32, tag="z1b")
            nc.vector.memset(zero1_b[:], 0.0)
            expb_b = const.tile([128, 1], F32, tag="expb")
            nc.vector.memset(expb_b[:], EXP_BIAS)
            g1b1_sb = const.tile([2, HID], F16, tag="g1b1")
            g2b2_sb = const.tile([2, HID], F16, tag="g2b2")
            nc.scalar.dma_start(g1b1_sb[:], g1b1[:])
            nc.scalar.dma_start(g2b2_sb[:], g2b2[:])

            # resident activations
            x16 = const.tile([128, HC, TPC], F16, tag="x16")
            q_sb = const.tile([128, 2, TOK], F8, tag="qsb")
            k_sb = const.tile([128, 2, TOK], F8, tag="ksb")
            v_sb = const.tile([128, TOK // 128, 256], F8, tag="vsb")
            at_all = const.tile([128, HC, TPC], F8, tag="atall")
            x2_sb = const.tile([128, HC, TPC], F16, tag="x2")

            # collective buffers: [slot/jj, partition, token] 3D layouts so
            # whole slots move in one descriptor-friendly DMA
            ag_in_a = dram.tile([8, 128, TPC], F8)
            ag_in_b = dram.tile([8, 128, TPC], F8)
            a2a_in_m = [dram.tile([NCORES, 128, TPC], F8, name=f"a2ai{m}")
                        for m in range(NH)]
            a2a_out_m = [dram.tile([NCORES, 128, TPC], F8, name=f"a2ao{m}")
                         for m in range(NH)]
            if single_core:
                ag_out_a = dram.tile([NCORES, 8, 128, TPC], F8)
                ag_out_b = dram.tile([NCORES, 8, 128, TPC], F8)
            else:
                ag_out_a = nc.dram_tensor(
                    "ag_out_a_sh", [NCORES, 8, 128, TPC], F8,
                    addr_space="Shared").ap()
                ag_out_b = nc.dram_tensor(
                    "ag_out_b_sh", [NCORES, 8, 128, TPC], F8,
                    addr_space="Shared").ap()

            def _ln_coeff(ps_sx, ps_sq, gb, put, lnp, psbc):
                mu = lnp.tile([1, TPC], F32, tag="mu", bufs=1)
                m2 = lnp.tile([1, TPC], F32, tag="m2", bufs=1)
                var = lnp.tile([1, TPC], F32, tag="var", bufs=1)
                lnv = lnp.tile([1, TPC], F32, tag="lnv", bufs=1)
                rstd16 = lnp.tile([1, TPC], F16, tag="rstd", bufs=1)
                mrs_ones = lnp.tile([2, TPC], F16, tag="mrso", bufs=1)
                nc.vector.tensor_scalar_mul(mu[:], ps_sx[:], 1.0 / HID)
                nc.vector.tensor_scalar_mul(m2[:], ps_sq[:], 1.0 / HID)
                nc.vector.tensor_mul(var[:], mu[:], mu[:])
                nc.vector.tensor_sub(var[:], m2[:], var[:])
                nc.scalar.activation(lnv[:], var[:], AF.Ln, bias=eps_b[:])
                nc.scalar.activation(rstd16[:], lnv[:], AF.Exp,
                                     bias=zero1_b[:], scale=-0.5)
                nc.vector.memset(mrs_ones[:], 1.0)
                nc.vector.tensor_mul(mrs_ones[0:1, :], mu[:], rstd16[:])
                nc.vector.tensor_scalar_mul(mrs_ones[0:1, :],
                                            mrs_ones[0:1, :], -1.0)
                for j in range(HC):
                    cs = slice(j * 128, (j + 1) * 128)
                    ps_c1 = psbc.tile([128, TPC], F32, tag="bc")
                    ps_c0 = psbc.tile([128, TPC], F32, tag="bc")
                    nc.tensor.matmul(ps_c1[:], gb[0:1, cs], rstd16[:],
                                     start=True, stop=True)
                    nc.tensor.matmul(ps_c0[:], gb[0:2, cs], mrs_ones[:],
                                     start=True, stop=True)
                    put(j, ps_c1, ps_c0)

            # ---------------- Stage A: load x, LN1, AllGather ----------------
            with (
                tc.tile_pool(name="lnA", bufs=4) as lnA,
                tc.tile_pool(name="psstA", bufs=2, space="PSUM") as psstA,
                tc.tile_pool(name="psbcA", bufs=4, space="PSUM") as psbcA,
            ):
                for g in range(4):
                    nc.sync.dma_start(
                        x16[:, 4 * g:4 * g + 4, :],
                        xT[4 * g:4 * g + 4, :, :].transpose([1, 0, 2]))
                ps_sx = psstA.tile([1, TPC], F32, tag="st")
                ps_sq = psstA.tile([1, TPC], F32, tag="st")
                sq_tiles = {}
                LAG = 3

                def stats1(j):
                    nc.tensor.matmul(ps_sx[:], ones16[:], x16[:, j, :],
                                     start=(j == 0), stop=(j == HC - 1))
                    nc.tensor.matmul(ps_sq[:], ones16[:], sq_tiles.pop(j)[:],
                                     start=(j == 0), stop=(j == HC - 1))

                for j in range(HC):
                    sqt = lnA.tile([128, TPC], F16, tag="sqt")
                    nc.vector.tensor_mul(sqt[:], x16[:, j, :], x16[:, j, :])
                    sq_tiles[j] = sqt
                    if j >= LAG:
                        stats1(j - LAG)
                for j in range(HC - LAG, HC):
                    stats1(j)

                h1_pair = [None]

                def put_h1(j, ps_c1, ps_c0):
                    # GPSIMD can't read PSUM: DVE does the psum mul, Act
                    # copies c0 to SBUF, Pool does the SBUF-only add.
                    t1 = lnA.tile([128, TPC], F16, tag="t1")
                    c0s = lnA.tile([128, TPC], F16, tag="c0s")
                    nc.vector.tensor_mul(t1[:], x16[:, j, :], ps_c1[:])
                    nc.scalar.activation(c0s[:], ps_c0[:], AF.Copy)
                    if j % 2 == 0:
                        h1_pair[0] = lnA.tile([128, 2, TPC], F8, tag="h1",
                                              bufs=2, name="h1p")
                    h1 = h1_pair[0]
                    nc.gpsimd.tensor_add(h1[:, j % 2, :], t1[:], c0s[:])
                    if j % 2 == 1:
                        u = j // 2
                        half, uu = (ag_in_a, u) if u < 4 else (ag_in_b, u - 4)
                        nc.sync.dma_start(
                            half[2 * uu:2 * uu + 2, :, :].transpose(
                                [1, 0, 2]), h1[:])

                _ln_coeff(ps_sx, ps_sq, g1b1_sb, put_h1, lnA, psbcA)

            if single_core:
                # AllGather stand-in: quarter volume per rank slot (baseline
                # convention) as every-4th-partition-row full-width writes —
                # 512B lines, one DMA per buffer, strided range still covers
                # every slot for dependency tracking.
                nc.sync.dma_start(
                    ag_out_a[:, :, ::4, :],
                    ag_in_a[:, ::4, :].unsqueeze(0).to_broadcast(
                        (NCORES, 8, 32, TPC)))
                nc.sync.dma_start(
                    ag_out_b[:, :, ::4, :],
                    ag_in_b[:, ::4, :].unsqueeze(0).to_broadcast(
                        (NCORES, 8, 32, TPC)))
            else:
                nc.gpsimd.collective_compute(
                    "AllGather", mybir.AluOpType.bypass, replica_groups=rg,
                    ins=[ag_in_a.opt()], outs=[ag_out_a.opt()])
                nc.gpsimd.collective_compute(
                    "AllGather", mybir.AluOpType.bypass, replica_groups=rg,
                    ins=[ag_in_b.opt()], outs=[ag_out_b.opt()])

            # ---------------- Stages B+C: qkv + attention (woven) -------------
            SB = S // TPC   # 4 query blocks per batch
            with (
                tc.tile_pool(name="cp", bufs=4) as cp,
                tc.tile_pool(name="ptp", bufs=3) as ptp,
                tc.tile_pool(name="zip", bufs=2) as zip_,
            ):
                def attn_block(m, b, qb, n_fast, pools, at4):
                    """n_fast of the 8 kg-pairs go through the DVE+Pool
                    fast-exp pipeline, the rest through Act exp."""
                    pss_p, pso_p, psdn_p = pools
                    mr = slice(64 * m, 64 * (m + 1))
                    qsl = slice(b * S + qb * TPC, b * S + (qb + 1) * TPC)
                    pso = pso_p.tile([128, TPC], F32, tag="o")
                    psden = psdn_p.tile([32, TPC], F32, tag="dn")
                    for kg in range(8):
                        pss = pss_p.tile([128, 2, TPC], F32, tag="s")
                        for h_ in range(2):
                            kc = kg * 2 + h_
                            ksl = slice(b * S + kc * 128,
                                        b * S + (kc + 1) * 128)
                            nc.tensor.matmul(
                                pss[:, h_, :], k_sb[mr, :, ksl],
                                q_sb[mr, :, qsl],
                                start=True, stop=True, perf_mode=DR)
                        pt = ptp.tile([128, 2, TPC], F8, tag="pt")
                        if kg >= 8 - n_fast:
                            zi = zip_.tile([128, 2, TPC], I32, tag="zi")
                            nc.vector.tensor_scalar(
                                zi[:].opt(), pss[:].opt(), FE_SCALE, FE_BIAS,
                                MULT, ADD)
                            nc.gpsimd.tensor_copy(pt[:].opt(),
                                                  zi[:].opt().bitcast(F32))
                        else:
                            nc.scalar.activation(
                                pt[:].opt(), pss[:].opt(), AF.Exp,
                                scale=SCALE / 256.0, bias=expb_b[:])
                        nc.tensor.matmul(
                            psden[:], ones8q[:], pt[:],
                            start=(kg == 0), stop=(kg == 7), perf_mode=DR)
                        nc.tensor.matmul(
                            pso[:],
                            v_sb[:, b * 16 + 2 * kg:b * 16 + 2 * kg + 2,
                                 m * 128:(m + 1) * 128],
                            pt[:],
                            start=(kg == 0), stop=(kg == 7), perf_mode=DR)
                    rec16 = cp.tile([1, TPC], F16, tag="rec")
                    nc.vector.reciprocal(rec16[:], psden[0:1, :])
                    rb = cp.tile([128, TPC], F16, tag="rb")
                    nc.gpsimd.partition_broadcast(rb[:], rec16[:])
                    nc.vector.tensor_mul(at4[:, qb, :], pso[:], rb[:])
                    if qb == SB - 1:
                        nc.sync.dma_start(
                            a2a_in_m[m][b * SB:(b + 1) * SB, :, :].transpose(
                                [1, 0, 2]), at4[:])

                def a2a(m):
                    if single_core:
                        a2a_mid = dram.tile([NCORES, 128, TPC], F8,
                                            name=f"a2am{m}")
                        nc.sync.dma_start(a2a_mid[:].opt(),
                                          a2a_in_m[m][:].opt())
                        nc.sync.dma_start(a2a_out_m[m][:].opt(),
                                          a2a_mid[:].opt())
                    else:
                        nc.gpsimd.collective_compute(
                            "AllToAll", mybir.AluOpType.bypass,
                            replica_groups=rg,
                            ins=[a2a_in_m[m].opt()], outs=[a2a_out_m[m].opt()])

                with (
                    tc.tile_pool(name="bwt", bufs=1) as bwt,
                    tc.tile_pool(name="htc", bufs=4) as htc,
                    tc.tile_pool(name="qkpre", bufs=2) as qkpre,
                    tc.tile_pool(name="ropet", bufs=6) as ropet,
                    tc.tile_pool(name="pssA", bufs=2, space="PSUM") as pssA,
                    tc.tile_pool(name="psoA", bufs=1, space="PSUM") as psoA,
                    tc.tile_pool(name="psdnA", bufs=1, space="PSUM") as psdnA,
                    tc.tile_pool(name="psqk", bufs=1, space="PSUM") as psqk,
                    tc.tile_pool(name="psv", bufs=1, space="PSUM") as psv,
                ):
                    poolsA = (pssA, psoA, psdnA)
                    rC = bwt.tile([128, TOK], F16, tag="rC")
                    rS = bwt.tile([128, TOK], F16, tag="rS")
                    nc.scalar.dma_start(rC[:], ropeC[:])
                    nc.scalar.dma_start(rS[:], ropeS[:])
                    wq_sb = bwt.tile([128, HC, 2, 128], F8, tag="wq")
                    wk_sb = bwt.tile([128, HC, 2, 128], F8, tag="wk")
                    wv_sb = bwt.tile([128, HC, 256], F8, tag="wv")
                    nc.scalar.dma_start(wq_sb[:].opt(), wq[:])
                    nc.scalar.dma_start(wk_sb[:].opt(), wk[:])
                    nc.scalar.dma_start(wv_sb[:].opt(), wv[:])

                    def qkv_tb(tb):
                        # gathered h for this token block: one DMA per half
                        ta = htc.tile([128, 8, TPC], F8, tag="hta")
                        tb_ = htc.tile([128, 8, TPC], F8, tag="htb")
                        nc.sync.dma_start(
                            ta[:], ag_out_a[tb].transpose([1, 0, 2]))
                        nc.sync.dma_start(
                            tb_[:], ag_out_b[tb].transpose([1, 0, 2]))
                        hts = ([ta[:, 2 * u:2 * u + 2, :] for u in range(4)]
                               + [tb_[:, 2 * u:2 * u + 2, :]
                                  for u in range(4)])
                        tcols = slice(tb * TPC, (tb + 1) * TPC)
                        for (w_sb, dst) in ((wq_sb, q_sb), (wk_sb, k_sb)):
                            pre = qkpre.tile([128, 2, TPC], F16, tag="pre")
                            for ch in range(2):
                                ps = psqk.tile([128, TPC], F32, tag="qk")
                                for jp in range(8):
                                    nc.tensor.matmul(
                                        ps[:],
                                        w_sb[:, 2 * jp:2 * jp + 2, ch, :],
                                        hts[jp], start=(jp == 0),
                                        stop=(jp == 7), perf_mode=DR)
                                nc.scalar.activation(pre[:, ch, :], ps[:],
                                                     AF.Copy, scale=QS)
                            # RoPE: 4 muls on DVE, 2 fp8 combines on Pool
                            t1 = ropet.tile([128, TPC], F16, tag="t1")
                            t2 = ropet.tile([128, TPC], F16, tag="t2")
                            t3 = ropet.tile([128, TPC], F16, tag="t3")
                            t4 = ropet.tile([128, TPC], F16, tag="t4")
                            nc.vector.tensor_mul(t1[:], pre[:, 0, :],
                                                 rC[:, tcols])
                            nc.vector.tensor_mul(t2[:], pre[:, 1, :],
                                                 rS[:, tcols])
                            nc.vector.tensor_mul(t3[:], pre[:, 0, :],
                                                 rS[:, tcols])
                            nc.vector.tensor_mul(t4[:], pre[:, 1, :],
                                                 rC[:, tcols])
                            nc.gpsimd.tensor_sub(dst[:, 0, tcols], t1[:],
                                                 t2[:])
                            nc.gpsimd.tensor_add(dst[:, 1, tcols], t3[:],
                                                 t4[:])
                        for mt in range(4):
                            ps = psv.tile([128, 256], F32, tag="v")
                            for jp in range(8):
                                nc.tensor.matmul(
                                    ps[:],
                                    hts[jp][:, :, mt * 128:(mt + 1) * 128],
                                    wv_sb[:, 2 * jp:2 * jp + 2, :],
                                    start=(jp == 0), stop=(jp == 7),
                                    perf_mode=DR)
                            nc.scalar.activation(v_sb[:, tb * 4 + mt, :],
                                                 ps[:], AF.Copy, scale=QS)

                    for tb in range(4):
                        qkv_tb(tb)
                    # weave: batch-0 attention of head 0 x qkv for batch 1.
                    at4 = cp.tile([128, SB, TPC], F8, tag="at4", bufs=2)
                    for qb in range(SB):
                        attn_block(0, 0, qb, 4, poolsA, at4)
                        qkv_tb(4 + qb)
                # qkv pools closed: wider psum rings for remaining attention
                with (
                    tc.tile_pool(name="pssB", bufs=2, space="PSUM") as pssB,
                    tc.tile_pool(name="psoB", bufs=3, space="PSUM") as psoB,
                    tc.tile_pool(name="psdnB", bufs=1, space="PSUM") as psdnB,
                ):
                    poolsB = (pssB, psoB, psdnB)
                    at4 = cp.tile([128, SB, TPC], F8, tag="at4", bufs=2)
                    for qb in range(SB):
                        attn_block(0, 1, qb, 3, poolsB, at4)
                    a2a(0)
                    # prefetch: even at_all chunks + first wo pairs during m1
                    nc.sync.dma_start(
                        at_all[:, 0:8, :],
                        a2a_out_m[0][:].transpose([1, 0, 2]))
                    wo_tiles = {}
                    for mp in range(2):
                        ws = wop.tile([128, 2, HC, 128], F8, tag="wo")
                        nc.scalar.dma_start(
                            ws[:].opt(),
                            wo[2 * mp:2 * mp + 2].transpose([1, 0, 2]))
                        wo_tiles[mp] = ws
                    for b in range(B):
                        at4 = cp.tile([128, SB, TPC], F8, tag="at4", bufs=2)
                        for qb in range(SB):
                            attn_block(1, b, qb, 3, poolsB, at4)
                    a2a(1)

            # ------------- Stages D+E ----------------------------------------
            with tc.tile_pool(name="late", bufs=1) as late:
              h2x = late.tile([128, HC, 2, TPC], F8, tag="h2x")
              ffx = late.tile([128, FFC, 2, TPC], F8, tag="ffx")
              with (
                tc.tile_pool(name="wop2", bufs=3) as wop2,
                tc.tile_pool(name="lnD", bufs=4) as lnD,
                tc.tile_pool(name="pso2", bufs=3, space="PSUM") as pso2_p,
                tc.tile_pool(name="psstD", bufs=2, space="PSUM") as psstD,
                tc.tile_pool(name="psbcD", bufs=3, space="PSUM") as psbcD,
              ):
                nc.sync.dma_start(
                    at_all[:, 8:HC, :],
                    a2a_out_m[1][:].transpose([1, 0, 2]))
                ps_sx2 = psstD.tile([1, TPC], F32, tag="st")
                ps_sq2 = psstD.tile([1, TPC], F32, tag="st")
                sq2_tiles = {}
                LAG2 = 2

                def stats2(mo):
                    nc.tensor.matmul(ps_sx2[:], ones16[:], x2_sb[:, mo, :],
                                     start=(mo == 0), stop=(mo == HC - 1))
                    nc.tensor.matmul(ps_sq2[:], ones16[:],
                                     sq2_tiles.pop(mo)[:],
                                     start=(mo == 0), stop=(mo == HC - 1))

                for mo in range(HC):
                    mp, s = divmod(mo, 2)
                    if s == 0:
                        if mp in wo_tiles:
                            ws = wo_tiles.pop(mp)
                        else:
                            ws = wop2.tile([128, 2, HC, 128], F8, tag="wo2")
                            nc.scalar.dma_start(
                                ws[:].opt(),
                                wo[2 * mp:2 * mp + 2].transpose([1, 0, 2]))
                        cur_wo = ws
                    ps = pso2_p.tile([128, TPC], F32, tag="o2")
                    for jp in range(8):
                        nc.tensor.matmul(
                            ps[:], cur_wo[:, s, 2 * jp:2 * jp + 2, :],
                            at_all[:, 2 * jp:2 * jp + 2, :],
                            start=(jp == 0), stop=(jp == 7), perf_mode=DR)
                    # x2 = psum * 2^-16 + x  (wo 1024x, at 64x)
                    nc.vector.scalar_tensor_tensor(
                        x2_sb[:, mo, :], ps[:], 2.0 ** -16, x16[:, mo, :],
                        MULT, ADD)
                    sq2 = lnD.tile([128, TPC], F16, tag="sq2")
                    nc.gpsimd.tensor_mul(sq2[:], x2_sb[:, mo, :],
                                         x2_sb[:, mo, :])
                    sq2_tiles[mo] = sq2
                    if mo >= LAG2:
                        stats2(mo - LAG2)
                for mo in range(HC - LAG2, HC):
                    stats2(mo)

                def put_h2(j, ps_c1, ps_c0):
                    t1 = lnD.tile([128, TPC], F16, tag="t1")
                    c0s = lnD.tile([128, TPC], F16, tag="c0s")
                    h2f = lnD.tile([128, TPC], F16, tag="h2f")
                    nc.vector.tensor_mul(t1[:], x2_sb[:, j, :], ps_c1[:])
                    nc.scalar.activation(c0s[:], ps_c0[:], AF.Copy)
                    nc.gpsimd.tensor_add(h2f[:], t1[:], c0s[:])
                    if j % 2 == 0:
                        nc.scalar.activation(h2x[:, j, 0, :], h2f[:], AF.Copy)
                    else:
                        nc.vector.tensor_copy(h2x[:, j, 0, :], h2f[:])
                    nc.gpsimd.tensor_sub(h2x[:, j, 1, :], h2f[:],
                                         h2x[:, j, 0, :])

                _ln_coeff(ps_sx2, ps_sq2, g2b2_sb, put_h2, lnD, psbcD)

              # ------------- Stage E: MLP -------------------------------------
              with (
                tc.tile_pool(name="wf1p", bufs=3) as wf1p,
                tc.tile_pool(name="wf2p", bufs=3) as wf2p,
                tc.tile_pool(name="mlt", bufs=4) as mlt,
                tc.tile_pool(name="psf1", bufs=3, space="PSUM") as psf1_p,
                tc.tile_pool(name="psf2", bufs=3, space="PSUM") as psf2_p,
              ):
                wf1_tiles = {}

                def wf1_load(mp):
                    ws = wf1p.tile([128, 2, HC, 2, 128], F8, tag="wf1")
                    nc.scalar.dma_start(
                        ws[:].opt(),
                        wf1[2 * mp:2 * mp + 2].transpose([1, 0, 2]))
                    wf1_tiles[mp] = ws

                wf1_load(0)
                wf1_load(1)
                for mo in range(FFC):
                    mp, s = divmod(mo, 2)
                    if s == 0:
                        cur_wf1 = wf1_tiles.pop(mp)
                        if 2 * (mp + 2) < FFC:
                            wf1_load(mp + 2)
                    ps = psf1_p.tile([128, TPC], F32, tag="f1")
                    for jp in range(8):  # hi*hi
                        nc.tensor.matmul(
                            ps[:], cur_wf1[:, s, 2 * jp:2 * jp + 2, 1, :],
                            h2x[:, 2 * jp:2 * jp + 2, 0, :],
                            start=(jp == 0), stop=False, perf_mode=DR)
                    for j in range(HC):  # cross: Wl*Ah + Wh*Al
                        nc.tensor.matmul(
                            ps[:], cur_wf1[:, s, j, :, :], h2x[:, j, :, :],
                            start=False, stop=(j == HC - 1), perf_mode=DR)
                    ff16 = mlt.tile([128, TPC], F16, tag="ff16")
                    nc.scalar.activation(ff16[:], ps[:], AF.Gelu,
                                         scale=1.0 / WS)
                    eng, oeng = ((nc.gpsimd, nc.vector) if mo % 2 == 0
                                 else (nc.vector, nc.gpsimd))
                    eng.tensor_copy(ffx[:, mo, 0, :], ff16[:])
                    oeng.tensor_sub(ffx[:, mo, 1, :], ff16[:],
                                    ffx[:, mo, 0, :])
                out_pair = [None]
                for mo in range(HC):
                    ws = wf2p.tile([128, FFC, 2, 128], F8, tag="wf2")
                    nc.scalar.dma_start(ws[:].opt(), wf2[mo])
                    ps = psf2_p.tile([128, TPC], F32, tag="f2")
                    for jp in range(FFC // 2):  # hi*hi
                        nc.tensor.matmul(
                            ps[:], ws[:, 2 * jp:2 * jp + 2, 1, :],
                            ffx[:, 2 * jp:2 * jp + 2, 0, :],
                            start=(jp == 0), stop=False, perf_mode=DR)
                    for j in range(FFC):  # cross
                        nc.tensor.matmul(
                            ps[:], ws[:, j, :, :], ffx[:, j, :, :],
                            start=False, stop=(j == FFC - 1), perf_mode=DR)
                    if mo % 2 == 0:
                        out_pair[0] = mlt.tile([128, 2, TPC], F32, tag="ot",
                                               bufs=2, name="otp")
                    ot = out_pair[0]
                    nc.vector.scalar_tensor_tensor(
                        ot[:, mo % 2, :], ps[:], 1.0 / WS, x2_sb[:, mo, :],
                        MULT, ADD)
                    if mo % 2 == 1:
                        nc.sync.dma_start(
                            outT[mo - 1:mo + 1, :, :].transpose([1, 0, 2]),
                            ot[:])
    return nc


def _build():
    if "nc" in _CACHE:
        return _CACHE["nc"]
    nc = bacc.Bacc(
        "TRN2", target_bir_lowering=False, debug=False,
        enable_asserts=True, num_devices=NCORES,
    )
    _emit(nc)
    nc.compile()
    _CACHE["nc"] = nc
    return nc


def _q8(v):
    return np.asarray(v, np.float32).astype(E4)


def prepare_inputs(x, pe, w_qkv, w_out, w_fc1, w_fc2, g1, b1, g2, b2):
    x = np.asarray(x, np.float32)
    pe = np.asarray(pe, np.float32)
    w_qkv = np.asarray(w_qkv, np.float32)
    w_out = np.asarray(w_out, np.float32)
    w_fc1 = np.asarray(w_fc1, np.float32)
    w_fc2 = np.asarray(w_fc2, np.float32)

    xf = x.reshape(TOK, HID)
    ropeC = np.tile(pe[:, 0::2].T, (2, B)).astype(np.float16)   # [128, TOK]
    ropeS = np.tile(pe[:, 1::2].T, (2, B)).astype(np.float16)
    g1b1 = np.stack([np.asarray(g1, np.float32),
                     np.asarray(b1, np.float32)]).astype(np.float16)
    g2b2 = np.stack([np.asarray(g2, np.float32),
                     np.asarray(b2, np.float32)]).astype(np.float16)

    # out_proj: strips over reordered contraction chunks
    # jj<8 -> head 2*jj (from a2a[0]); jj>=8 -> head 2*(jj-8)+1 (a2a[1])
    head_of = [2 * jj if jj < 8 else 2 * (jj - 8) + 1 for jj in range(HC)]
    wo_h = np.empty((HC, 128, HC * 128), dtype=E4)
    wt = (w_out * WS).astype(np.float32)   # [out, feat]
    for mo in range(HC):
        blk = np.empty((128, HC, 128), np.float32)
        for jj in range(HC):
            h = head_of[jj]
            blk[:, jj, :] = wt[mo * 128:(mo + 1) * 128,
                               h * 128:(h + 1) * 128].T
        wo_h[mo] = _q8(blk.reshape(128, HC * 128))

    def hilo_strips(w, n_strips, kc):
        w = (w * WS).astype(np.float32)
        hi = _q8(w).astype(np.float32)
        lo = _q8(w - hi).astype(np.float32)
        out = np.empty((n_strips, 128, kc * 2 * 128), dtype=E4)
        for mo in range(n_strips):
            rows = slice(mo * 128, (mo + 1) * 128)
            blk = np.empty((128, kc, 2, 128), np.float32)
            wl = lo[rows]; wh = hi[rows]    # [128(col), kc*128]
            blk[:, :, 0, :] = wl.reshape(128, kc, 128).transpose(2, 1, 0)
            blk[:, :, 1, :] = wh.reshape(128, kc, 128).transpose(2, 1, 0)
            out[mo] = _q8(blk.reshape(128, kc * 2 * 128))
        return out

    wf1_h = hilo_strips(w_fc1, FFC, HC)     # [FFC, 128, HC*2*128]
    wf2_h = hilo_strips(w_fc2, HC, FFC)     # [HC, 128, FFC*2*128]

    in_maps = []
    for c in range(NCORES):
        hsl = slice(2 * c * D, (2 * c + 2) * D)

        def qk_lay(rows):
            # rows [256, HID] (2 heads) -> [128(p), HC(j), 2(ch), 128(m,pp)]
            r = (rows * WS).astype(np.float32)
            t = r.reshape(2, 64, 2, HC, 128)       # [m, pp, ch, j, p]
            t = t.transpose(4, 3, 2, 0, 1)          # [p, j, ch, m, pp]
            return _q8(t.reshape(128, HC * 2 * 128))

        qrows = w_qkv[hsl]
        krows = w_qkv[HID + 2 * c * D: HID + (2 * c + 2) * D]
        vrows = w_qkv[2 * HID + 2 * c * D: 2 * HID + (2 * c + 2) * D]
        vv = (vrows * WS).astype(np.float32).reshape(2, 128, HC, 128)
        wv_c = _q8(vv.transpose(3, 2, 0, 1).reshape(128, HC * 256))

        xTc = np.ascontiguousarray(
            xf[c * TPC:(c + 1) * TPC].T).astype(np.float16)
        in_maps.append({
            "xT": xTc.reshape(HC, 128, TPC),
            "wq": qk_lay(qrows), "wk": qk_lay(krows), "wv": wv_c,
            "wo": wo_h, "wf1": wf1_h, "wf2": wf2_h,
            "g1b1": g1b1, "g2b2": g2b2,
            "ropeC": ropeC, "ropeS": ropeS,
        })
    return in_maps


def run(in_maps, **kwargs):
    nc = _build()
    return bass_utils.run_bass_kernel_spmd(
        nc, in_maps, core_ids=list(range(NCORES)), **kwargs
    )


def kernel(x, pe, w_qkv, w_out, w_fc1, w_fc2, g1, b1, g2, b2):
    in_maps = prepare_inputs(x, pe, w_qkv, w_out, w_fc1, w_fc2, g1, b1, g2, b2)
    res = run(in_maps)
    fullT = np.concatenate(
        [res.results[c]["outT"].reshape(HID, TPC) for c in range(NCORES)],
        axis=1)
    return np.ascontiguousarray(fullT.T).reshape(B, S, HID).astype(np.float32)


# revision 36
# speedup vs baseline: 1.5983x; 1.0961x over previous
"""MiniTransformerLayer on 8 Trainium2 NeuronCores — fp8 DoubleRow edition.

Sharding (as baseline): core c owns tokens [512c, 512(c+1)) and heads
{2c, 2c+1}; 2 AllGathers (LN1 out, fp8) + 2 AllToAlls (attn out, fp8).

Numerics:
  - all attention-side matmuls in fp8e4 DoubleRow (2 K-chunks per
    instruction, 0.5 cyc/row): qkv, scores (64-partition DR with heads
    stacked in partitions 0:64/64:128), attn@V, softmax denominator
    (ones-DR), out_proj.
  - MLP fc1/fc2 as 3-term hi/lo fp8 DR: W*A = Wh*Ah + (Wl*Ah + Wh*Al),
    weights hi/lo prepared on host, activations hi/lo on device.
  - softmax exp split across engines: Act native exp, plus a
    Schraudolph exp2 bit-trick pipeline (DVE f32->int32 convert from
    PSUM, Pool bitcast copy to fp8). The denominator cancels the shared
    approximation bias; measured end-to-end rel err ~4e-3.
  - LayerNorm stats via fp16 ones-matmuls; per-chunk scale/offset via
    rank-1/rank-2 coefficient matmuls (g,b folded).
  - fixed power-of-2 scales: weights x1024 (host), q/k/v fp8 at 16x,
    probs at 1x, attn out at 64x; unscaled in Act copies / stt epilogues.

Schedule notes: every dma_start costs ~630ns of serialized HWDGE
descriptor-generation, so transfers are batched through 3D DRAM
tensors + transposed access patterns (gathered h: 2 DMAs per token
block; attention outputs: 1 DMA per (head, batch); weights: 2 strips
per DMA). qkv for batch-1 token blocks is woven into the (head0,
batch0) attention blocks; LN stats matmuls lag their elementwise
producers to stay off the critical path.
"""

import sys

sys.path.insert(0, "/opt/trn_rl_repo")

import numpy as np
import ml_dtypes

import concourse.bass as bass
import concourse.bacc as bacc
import concourse.tile as tile
import concourse.mybir as mybir
from concourse import bass_utils

F8 = mybir.dt.float8e4
F16 = mybir.dt.float16
F32 = mybir.dt.float32
I32 = mybir.dt.int32
AF = mybir.ActivationFunctionType
DR = mybir.MatmulPerfMode.DoubleRow
E4 = ml_dtypes.float8_e4m3

NCORES = 8
B, S, HID, HEADS, D, FFN = 2, 2048, 2048, 16, 128, 4096
TOK = B * S            # 4096 flat tokens
TPC = TOK // NCORES    # 512 tokens per core
HC = HID // 128        # 16 hidden chunks
FFC = FFN // 128       # 32 ffn chunks
NH = HEADS // NCORES   # 2 heads per core
SCALE = 1.0 / float(np.sqrt(D))
EXP_BIAS = -3.0
EPS = 1e-5
WS = 1024.0            # host weight scale
QS = 2.0 ** -6         # psum(1024 q) -> 16 q
MULT, ADD, SUB = (mybir.AluOpType.mult, mybir.AluOpType.add,
                  mybir.AluOpType.subtract)
# Schraudolph exp2 bit trick: int32 bits = trunc(z*A + BC) ~ exp(z) bits
EXP_A = 12102203.161561485          # 2^23 / ln2
EXP_BC = 1064866805.0               # 127*2^23 - 366393
FE_SCALE = EXP_A * SCALE / 256.0
FE_BIAS = EXP_BC + EXP_A * EXP_BIAS

_CACHE = {}


def _emit(nc, single_core=False):
    xT = nc.dram_tensor("xT", [HC, 128, TPC], F16, kind="ExternalInput")
    wq = nc.dram_tensor("wq", [128, HC * 2 * 128], F8, kind="ExternalInput")
    wk = nc.dram_tensor("wk", [128, HC * 2 * 128], F8, kind="ExternalInput")
    wv = nc.dram_tensor("wv", [128, HC * 256], F8, kind="ExternalInput")
    wo = nc.dram_tensor("wo", [HC, 128, HC * 128], F8, kind="ExternalInput")
    wf1 = nc.dram_tensor("wf1", [FFC, 128, HC * 2 * 128], F8,
                         kind="ExternalInput")
    wf2 = nc.dram_tensor("wf2", [HC, 128, FFC * 2 * 128], F8,
                         kind="ExternalInput")
    g1b1 = nc.dram_tensor("g1b1", [2, HID], F16, kind="ExternalInput")
    g2b2 = nc.dram_tensor("g2b2", [2, HID], F16, kind="ExternalInput")
    ropeC = nc.dram_tensor("ropeC", [128, TOK], F16, kind="ExternalInput")
    ropeS = nc.dram_tensor("ropeS", [128, TOK], F16, kind="ExternalInput")
    outT = nc.dram_tensor("outT", [HC, 128, TPC], F32, kind="ExternalOutput")

    rg = [list(range(NCORES))]

    with tile.TileContext(nc) as tc:
        with (
            nc.allow_low_precision(reason="fp8 kernel: quantized by design"),
            tc.tile_pool(name="const", bufs=1) as const,
            tc.tile_pool(name="dram", bufs=1, space="DRAM") as dram,
            tc.tile_pool(name="wop", bufs=2) as wop,
        ):
            ones16 = const.tile([128, 1], F16, tag="on16")
            nc.vector.memset(ones16[:], 1.0)
            ones8q = const.tile([128, 2, 32], F8, tag="on8q")
            nc.vector.memset(ones8q[:], 0.25)
            eps_b = const.tile([1, 1], F32, tag="epsb")
            nc.vector.memset(eps_b[:], EPS)
            zero1_b = const.tile([1, 1], F32, tag="z1b")
            nc.vector.memset(zero1_b[:], 0.0)
            expb_b = const.tile([128, 1], F32, tag="expb")
            nc.vector.memset(expb_b[:], EXP_BIAS)
            g1b1_sb = const.tile([2, HID], F16, tag="g1b1")
            g2b2_sb = const.tile([2, HID], F16, tag="g2b2")
            nc.scalar.dma_start(g1b1_sb[:], g1b1[:])
            nc.scalar.dma_start(g2b2_sb[:], g2b2[:])

            # resident activations
            x16 = const.tile([128, HC, TPC], F16, tag="x16")
            q_sb = const.tile([128, 2, TOK], F8, tag="qsb")
            k_sb = const.tile([128, 2, TOK], F8, tag="ksb")
            v_sb = const.tile([128, TOK // 128, 256], F8, tag="vsb")
            at_all = const.tile([128, HC, TPC], F8, tag="atall")
            x2_sb = const.tile([128, HC, TPC], F16, tag="x2")

            # collective buffers: [slot/jj, partition, token] 3D layouts so
            # whole slots move in one descriptor-friendly DMA
            ag_in_a = dram.tile([8, 128, TPC], F8)
            ag_in_b = dram.tile([8, 128, TPC], F8)
            a2a_in_m = [dram.tile([NCORES, 128, TPC], F8, name=f"a2ai{m}")
                        for m in range(NH)]
            a2a_out_m = [dram.tile([NCORES, 128, TPC], F8, name=f"a2ao{m}")
                         for m in range(NH)]
            if single_core:
                ag_out_a = dram.tile([NCORES, 8, 128, TPC], F8)
                ag_out_b = dram.tile([NCORES, 8, 128, TPC], F8)
            else:
                ag_out_a = nc.dram_tensor(
                    "ag_out_a_sh", [NCORES, 8, 128, TPC], F8,
                    addr_space="Shared").ap()
                ag_out_b = nc.dram_tensor(
                    "ag_out_b_sh", [NCORES, 8, 128, TPC], F8,
                    addr_space="Shared").ap()

            def _ln_coeff(ps_sx, ps_sq, gb, put, lnp, psbc):
                mu = lnp.tile([1, TPC], F32, tag="mu", bufs=1)
                m2 = lnp.tile([1, TPC], F32, tag="m2", bufs=1)
                var = lnp.tile([1, TPC], F32, tag="var", bufs=1)
                lnv = lnp.tile([1, TPC], F32, tag="lnv", bufs=1)
                rstd16 = lnp.tile([1, TPC], F16, tag="rstd", bufs=1)
                mrs_ones = lnp.tile([2, TPC], F16, tag="mrso", bufs=1)
                nc.vector.tensor_scalar_mul(mu[:], ps_sx[:], 1.0 / HID)
                nc.vector.tensor_scalar_mul(m2[:], ps_sq[:], 1.0 / HID)
                nc.vector.tensor_mul(var[:], mu[:], mu[:])
                nc.vector.tensor_sub(var[:], m2[:], var[:])
                nc.scalar.activation(lnv[:], var[:], AF.Ln, bias=eps_b[:])
                nc.scalar.activation(rstd16[:], lnv[:], AF.Exp,
                                     bias=zero1_b[:], scale=-0.5)
                nc.vector.memset(mrs_ones[:], 1.0)
                nc.vector.tensor_mul(mrs_ones[0:1, :], mu[:], rstd16[:])
                nc.vector.tensor_scalar_mul(mrs_ones[0:1, :],
                                            mrs_ones[0:1, :], -1.0)
                for j in range(HC):
                    cs = slice(j * 128, (j + 1) * 128)
                    ps_c1 = psbc.tile([128, TPC], F32, tag="bc")
                    ps_c0 = psbc.tile([128, TPC], F32, tag="bc")
                    nc.tensor.matmul(ps_c1[:], gb[0:1, cs], rstd16[:],
                                     start=True, stop=True)
                    nc.tensor.matmul(ps_c0[:], gb[0:2, cs], mrs_ones[:],
                                     start=True, stop=True)
                    put(j, ps_c1, ps_c0)

            # ---------------- Stage A: load x, LN1, AllGather ----------------
            with (
                tc.tile_pool(name="lnA", bufs=4) as lnA,
                tc.tile_pool(name="psstA", bufs=2, space="PSUM") as psstA,
                tc.tile_pool(name="psbcA", bufs=4, space="PSUM") as psbcA,
            ):
                for g in range(4):
                    nc.sync.dma_start(
                        x16[:, 4 * g:4 * g + 4, :],
                        xT[4 * g:4 * g + 4, :, :].transpose([1, 0, 2]))
                ps_sx = psstA.tile([1, TPC], F32, tag="st")
                ps_sq = psstA.tile([1, TPC], F32, tag="st")
                sq_tiles = {}
                LAG = 3

                def stats1(j):
                    nc.tensor.matmul(ps_sx[:], ones16[:], x16[:, j, :],
                                     start=(j == 0), stop=(j == HC - 1))
                    nc.tensor.matmul(ps_sq[:], ones16[:], sq_tiles.pop(j)[:],
                                     start=(j == 0), stop=(j == HC - 1))

                for j in range(HC):
                    sqt = lnA.tile([128, TPC], F16, tag="sqt")
                    nc.vector.tensor_mul(sqt[:], x16[:, j, :], x16[:, j, :])
                    sq_tiles[j] = sqt
                    if j >= LAG:
                        stats1(j - LAG)
                for j in range(HC - LAG, HC):
                    stats1(j)

                h1_pair = [None]

                def put_h1(j, ps_c1, ps_c0):
                    # GPSIMD can't read PSUM: DVE does the psum mul, Act
                    # copies c0 to SBUF, Pool does the SBUF-only add.
                    t1 = lnA.tile([128, TPC], F16, tag="t1")
                    c0s = lnA.tile([128, TPC], F16, tag="c0s")
                    nc.vector.tensor_mul(t1[:], x16[:, j, :], ps_c1[:])
                    nc.scalar.activation(c0s[:], ps_c0[:], AF.Copy)
                    if j % 2 == 0:
                        h1_pair[0] = lnA.tile([128, 2, TPC], F8, tag="h1",
                                              bufs=2, name="h1p")
                    h1 = h1_pair[0]
                    nc.gpsimd.tensor_add(h1[:, j % 2, :], t1[:], c0s[:])
                    if j % 2 == 1:
                        u = j // 2
                        half, uu = (ag_in_a, u) if u < 4 else (ag_in_b, u - 4)
                        nc.sync.dma_start(
                            half[2 * uu:2 * uu + 2, :, :].transpose(
                                [1, 0, 2]), h1[:])

                _ln_coeff(ps_sx, ps_sq, g1b1_sb, put_h1, lnA, psbcA)

            if single_core:
                # AllGather stand-in: quarter volume per rank slot (baseline
                # convention) as every-4th-partition-row full-width writes —
                # 512B lines, one DMA per buffer, strided range still covers
                # every slot for dependency tracking.
                for r in range(0, NCORES, 2):
                    nc.sync.dma_start(
                        ag_out_a[r:r + 2, :, ::4, :],
                        ag_in_a[:, ::4, :].unsqueeze(0).to_broadcast(
                            (2, 8, 32, TPC)))
                    nc.sync.dma_start(
                        ag_out_b[r:r + 2, :, ::4, :],
                        ag_in_b[:, ::4, :].unsqueeze(0).to_broadcast(
                            (2, 8, 32, TPC)))
            else:
                nc.gpsimd.collective_compute(
                    "AllGather", mybir.AluOpType.bypass, replica_groups=rg,
                    ins=[ag_in_a.opt()], outs=[ag_out_a.opt()])
                nc.gpsimd.collective_compute(
                    "AllGather", mybir.AluOpType.bypass, replica_groups=rg,
                    ins=[ag_in_b.opt()], outs=[ag_out_b.opt()])

            # ---------------- Stages B+C: qkv + attention (woven) -------------
            SB = S // TPC   # 4 query blocks per batch
            with (
                tc.tile_pool(name="cp", bufs=4) as cp,
                tc.tile_pool(name="ptp", bufs=3) as ptp,
                tc.tile_pool(name="zip", bufs=2) as zip_,
            ):
                def attn_block(m, b, qb, fast_pairs, pools, at4):
                    """fast_pairs kg-indices go through the DVE(+Pool)
                    fast-exp pipeline, the rest through Act exp. den/pv
                    matmuls are emitted 2 pairs behind the scores so the
                    in-order PE never waits on an in-flight exp."""
                    pss_p, pso_p, psdn_p = pools
                    mr = slice(64 * m, 64 * (m + 1))
                    qsl = slice(b * S + qb * TPC, b * S + (qb + 1) * TPC)
                    pso = pso_p.tile([128, TPC], F32, tag="o")
                    psden = psdn_p.tile([32, TPC], F32, tag="dn")
                    pts = {}

                    def score_exp(kg, pool_copy):
                        pss = pss_p.tile([128, 2, TPC], F32, tag="s")
                        for h_ in range(2):
                            kc = kg * 2 + h_
                            ksl = slice(b * S + kc * 128,
                                        b * S + (kc + 1) * 128)
                            nc.tensor.matmul(
                                pss[:, h_, :], k_sb[mr, :, ksl],
                                q_sb[mr, :, qsl],
                                start=True, stop=True, perf_mode=DR)
                        pt = ptp.tile([128, 2, TPC], F8, tag="pt", bufs=6)
                        pts[kg] = pt
                        if kg in fast_pairs:
                            zi = zip_.tile([128, 2, TPC], I32, tag="ziw",
                                           bufs=3)
                            nc.vector.tensor_scalar(
                                zi[:].opt(), pss[:].opt(), FE_SCALE, FE_BIAS,
                                MULT, ADD)
                            ceng = nc.gpsimd if pool_copy else nc.vector
                            ceng.tensor_copy(pt[:].opt(),
                                             zi[:].opt().bitcast(F32))
                        else:
                            nc.scalar.activation(
                                pt[:].opt(), pss[:].opt(), AF.Exp,
                                scale=SCALE / 256.0, bias=expb_b[:])

                    def den_pv(kg):
                        pt = pts.pop(kg)
                        nc.tensor.matmul(
                            psden[:], ones8q[:], pt[:],
                            start=(kg == 0), stop=(kg == 7), perf_mode=DR)
                        nc.tensor.matmul(
                            pso[:],
                            v_sb[:, b * 16 + 2 * kg:b * 16 + 2 * kg + 2,
                                 m * 128:(m + 1) * 128],
                            pt[:],
                            start=(kg == 0), stop=(kg == 7), perf_mode=DR)

                    nf = 0
                    for kg in range(8):
                        if kg in fast_pairs:
                            nf += 1
                        score_exp(kg, pool_copy=(nf % 2 == 1))
                        if kg >= 2:
                            den_pv(kg - 2)
                    den_pv(6)
                    den_pv(7)
                    rec16 = cp.tile([1, TPC], F16, tag="rec")
                    nc.vector.reciprocal(rec16[:], psden[0:1, :])
                    rb = cp.tile([128, TPC], F16, tag="rb")
                    nc.gpsimd.partition_broadcast(rb[:], rec16[:])
                    nc.vector.tensor_mul(at4[:, qb, :], pso[:], rb[:])
                    if qb == SB - 1:
                        nc.sync.dma_start(
                            a2a_in_m[m][b * SB:(b + 1) * SB, :, :].transpose(
                                [1, 0, 2]), at4[:])

                def attn_two(m, b, qb0, fast_pairs, pools, at4):
                    """Interleave two query blocks so engines always have
                    independent work between dependency hops."""
                    pss_p, pso_p, psdn_p = pools
                    mr = slice(64 * m, 64 * (m + 1))
                    ctx = []
                    for qb in (qb0, qb0 + 1):
                        qsl = slice(b * S + qb * TPC, b * S + (qb + 1) * TPC)
                        pso = pso_p.tile([128, TPC], F32, tag="o")
                        psden = psdn_p.tile([32, TPC], F32, tag="dn")
                        ctx.append((qb, qsl, pso, psden, {}))

                    cnt = [0]

                    def score_exp(ci, kg):
                        # per-kc pss tiles (1 PSUM bank each) keep the
                        # scores->exp ring turning at full rate
                        qb, qsl, pso, psden, pts = ctx[ci]
                        pt = ptp.tile([128, 2, TPC], F8, tag="pt", bufs=6)
                        pts[kg] = pt
                        for h_ in range(2):
                            kc = kg * 2 + h_
                            ksl = slice(b * S + kc * 128,
                                        b * S + (kc + 1) * 128)
                            pss = pss_p.tile([128, TPC], F32, tag="s")
                            nc.tensor.matmul(
                                pss[:], k_sb[mr, :, ksl], q_sb[mr, :, qsl],
                                start=True, stop=True, perf_mode=DR)
                            cnt[0] += 1
                            if kg in fast_pairs:
                                zi = zip_.tile([128, TPC], I32, tag="zi",
                                               bufs=4)
                                nc.vector.tensor_scalar(
                                    zi[:], pss[:], FE_SCALE, FE_BIAS,
                                    MULT, ADD)
                                ceng = (nc.vector if cnt[0] % 4 == 0
                                        else nc.gpsimd)
                                ceng.tensor_copy(pt[:, h_, :],
                                                 zi[:].bitcast(F32))
                            else:
                                nc.scalar.activation(
                                    pt[:, h_, :], pss[:], AF.Exp,
                                    scale=SCALE / 256.0, bias=expb_b[:])

                    def den_pv(ci, kg):
                        qb, qsl, pso, psden, pts = ctx[ci]
                        pt = pts.pop(kg)
                        nc.tensor.matmul(
                            psden[:], ones8q[:], pt[:],
                            start=(kg == 0), stop=(kg == 7), perf_mode=DR)
                        nc.tensor.matmul(
                            pso[:],
                            v_sb[:, b * 16 + 2 * kg:b * 16 + 2 * kg + 2,
                                 m * 128:(m + 1) * 128],
                            pt[:],
                            start=(kg == 0), stop=(kg == 7), perf_mode=DR)

                    for kg in range(8):
                        for ci in range(2):
                            score_exp(ci, kg)
                        if kg >= 2:
                            den_pv(0, kg - 2)
                            den_pv(1, kg - 2)
                    for kg in (6, 7):
                        den_pv(0, kg)
                        den_pv(1, kg)
                    for ci in range(2):
                        qb, qsl, pso, psden, pts = ctx[ci]
                        rec16 = cp.tile([1, TPC], F16, tag="rec")
                        nc.vector.reciprocal(rec16[:], psden[0:1, :])
                        rb = cp.tile([128, TPC], F16, tag="rb")
                        nc.gpsimd.partition_broadcast(rb[:], rec16[:])
                        nc.vector.tensor_mul(at4[:, qb, :], pso[:], rb[:])
                        if qb == SB - 1:
                            nc.sync.dma_start(
                                a2a_in_m[m][b * SB:(b + 1) * SB, :, :]
                                .transpose([1, 0, 2]), at4[:])

                def a2a(m):
                    if single_core:
                        # two-hop stand-in (baseline convention), halves
                        # pipelined so wire transfers overlap
                        a2a_mid = dram.tile([NCORES, 128, TPC], F8,
                                            name=f"a2am{m}")
                        for hh in range(2):
                            sl = slice(4 * hh, 4 * hh + 4)
                            nc.sync.dma_start(a2a_mid[sl].opt(),
                                              a2a_in_m[m][sl].opt())
                        for hh in range(2):
                            sl = slice(4 * hh, 4 * hh + 4)
                            nc.sync.dma_start(a2a_out_m[m][sl].opt(),
                                              a2a_mid[sl].opt())
                    else:
                        nc.gpsimd.collective_compute(
                            "AllToAll", mybir.AluOpType.bypass,
                            replica_groups=rg,
                            ins=[a2a_in_m[m].opt()], outs=[a2a_out_m[m].opt()])

                with (
                    tc.tile_pool(name="bwt", bufs=1) as bwt,
                    tc.tile_pool(name="htc", bufs=4) as htc,
                    tc.tile_pool(name="qkpre", bufs=2) as qkpre,
                    tc.tile_pool(name="ropet", bufs=4) as ropet,
                    tc.tile_pool(name="pssA", bufs=2, space="PSUM") as pssA,
                    tc.tile_pool(name="psoA", bufs=1, space="PSUM") as psoA,
                    tc.tile_pool(name="psdnA", bufs=1, space="PSUM") as psdnA,
                    tc.tile_pool(name="psqk", bufs=1, space="PSUM") as psqk,
                    tc.tile_pool(name="psv", bufs=1, space="PSUM") as psv,
                ):
                    poolsA = (pssA, psoA, psdnA)
                    rC = bwt.tile([128, TOK], F16, tag="rC")
                    rS = bwt.tile([128, TOK], F16, tag="rS")
                    nc.scalar.dma_start(rC[:], ropeC[:])
                    nc.scalar.dma_start(rS[:], ropeS[:])
                    wq_sb = bwt.tile([128, HC, 2, 128], F8, tag="wq")
                    wk_sb = bwt.tile([128, HC, 2, 128], F8, tag="wk")
                    wv_sb = bwt.tile([128, HC, 256], F8, tag="wv")
                    nc.scalar.dma_start(wq_sb[:].opt(), wq[:])
                    nc.scalar.dma_start(wk_sb[:].opt(), wk[:])
                    nc.scalar.dma_start(wv_sb[:].opt(), wv[:])

                    def qkv_tb(tb):
                        # gathered h for this token block: one DMA per half
                        ta = htc.tile([128, 8, TPC], F8, tag="hta")
                        tb_ = htc.tile([128, 8, TPC], F8, tag="htb")
                        nc.sync.dma_start(
                            ta[:], ag_out_a[tb].transpose([1, 0, 2]))
                        nc.sync.dma_start(
                            tb_[:], ag_out_b[tb].transpose([1, 0, 2]))
                        hts = ([ta[:, 2 * u:2 * u + 2, :] for u in range(4)]
                               + [tb_[:, 2 * u:2 * u + 2, :]
                                  for u in range(4)])
                        tcols = slice(tb * TPC, (tb + 1) * TPC)
                        for (w_sb, dst) in ((wq_sb, q_sb), (wk_sb, k_sb)):
                            pre = qkpre.tile([128, 2, TPC], F16, tag="pre")
                            for ch in range(2):
                                ps = psqk.tile([128, TPC], F32, tag="qk")
                                for jp in range(8):
                                    nc.tensor.matmul(
                                        ps[:],
                                        w_sb[:, 2 * jp:2 * jp + 2, ch, :],
                                        hts[jp], start=(jp == 0),
                                        stop=(jp == 7), perf_mode=DR)
                                nc.scalar.activation(pre[:, ch, :], ps[:],
                                                     AF.Copy, scale=QS)
                            # RoPE: 4 muls on DVE, 2 fp8 combines on Pool
                            t1 = ropet.tile([128, TPC], F16, tag="t1")
                            t2 = ropet.tile([128, TPC], F16, tag="t2")
                            t3 = ropet.tile([128, TPC], F16, tag="t3")
                            t4 = ropet.tile([128, TPC], F16, tag="t4")
                            nc.vector.tensor_mul(t1[:], pre[:, 0, :],
                                                 rC[:, tcols])
                            nc.vector.tensor_mul(t2[:], pre[:, 1, :],
                                                 rS[:, tcols])
                            nc.vector.tensor_mul(t3[:], pre[:, 0, :],
                                                 rS[:, tcols])
                            nc.vector.tensor_mul(t4[:], pre[:, 1, :],
                                                 rC[:, tcols])
                            nc.gpsimd.tensor_sub(dst[:, 0, tcols], t1[:],
                                                 t2[:])
                            nc.gpsimd.tensor_add(dst[:, 1, tcols], t3[:],
                                                 t4[:])
                        for mt in range(4):
                            ps = psv.tile([128, 256], F32, tag="v")
                            for jp in range(8):
                                nc.tensor.matmul(
                                    ps[:],
                                    hts[jp][:, :, mt * 128:(mt + 1) * 128],
                                    wv_sb[:, 2 * jp:2 * jp + 2, :],
                                    start=(jp == 0), stop=(jp == 7),
                                    perf_mode=DR)
                            nc.scalar.activation(v_sb[:, tb * 4 + mt, :],
                                                 ps[:], AF.Copy, scale=QS)

                    for tb in range(4):
                        qkv_tb(tb)
                    # weave: batch-0 attention of head 0 x qkv for batch 1.
                    at4 = cp.tile([128, SB, TPC], F8, tag="at4", bufs=2)
                    for qb in range(SB):
                        attn_block(0, 0, qb, {2, 5}, poolsA, at4)
                        qkv_tb(4 + qb)
                # qkv pools closed: wider psum rings for remaining attention
                with (
                    tc.tile_pool(name="pssB", bufs=4, space="PSUM") as pssB,
                    tc.tile_pool(name="psoB", bufs=2, space="PSUM") as psoB,
                    tc.tile_pool(name="psdnB", bufs=2, space="PSUM") as psdnB,
                ):
                    poolsB = (pssB, psoB, psdnB)
                    at4 = cp.tile([128, SB, TPC], F8, tag="at4", bufs=2)
                    for qb in (0, 2):
                        attn_two(0, 1, qb, {1, 3, 5}, poolsB, at4)
                    a2a(0)
                    # prefetch: even at_all chunks + first wo pairs during m1
                    for hh in range(2):
                        nc.sync.dma_start(
                            at_all[:, 4 * hh:4 * hh + 4, :],
                            a2a_out_m[0][4 * hh:4 * hh + 4].transpose(
                                [1, 0, 2]))
                    wo_tiles = {}
                    for mp in range(2):
                        ws = wop.tile([128, 2, HC, 128], F8, tag="wo")
                        nc.scalar.dma_start(
                            ws[:].opt(),
                            wo[2 * mp:2 * mp + 2].transpose([1, 0, 2]))
                        wo_tiles[mp] = ws
                    for b in range(B):
                        at4 = cp.tile([128, SB, TPC], F8, tag="at4", bufs=2)
                        for qb in (0, 2):
                            attn_two(1, b, qb, {1, 3, 5}, poolsB, at4)
                    a2a(1)

            # ------------- Stages D+E ----------------------------------------
            with tc.tile_pool(name="late", bufs=1) as late:
              h2x = late.tile([128, HC, 2, TPC], F8, tag="h2x")
              ffx = late.tile([128, FFC, 2, TPC], F8, tag="ffx")
              with (
                tc.tile_pool(name="wop2", bufs=3) as wop2,
                tc.tile_pool(name="lnD", bufs=4) as lnD,
                tc.tile_pool(name="pso2", bufs=3, space="PSUM") as pso2_p,
                tc.tile_pool(name="psstD", bufs=2, space="PSUM") as psstD,
                tc.tile_pool(name="psbcD", bufs=3, space="PSUM") as psbcD,
              ):
                for hh in range(2):
                    nc.sync.dma_start(
                        at_all[:, 8 + 4 * hh:12 + 4 * hh, :],
                        a2a_out_m[1][4 * hh:4 * hh + 4].transpose([1, 0, 2]))
                ps_sx2 = psstD.tile([1, TPC], F32, tag="st")
                ps_sq2 = psstD.tile([1, TPC], F32, tag="st")
                sq2_tiles = {}
                LAG2 = 2

                def stats2(mo):
                    nc.tensor.matmul(ps_sx2[:], ones16[:], x2_sb[:, mo, :],
                                     start=(mo == 0), stop=(mo == HC - 1))
                    nc.tensor.matmul(ps_sq2[:], ones16[:],
                                     sq2_tiles.pop(mo)[:],
                                     start=(mo == 0), stop=(mo == HC - 1))

                for mo in range(HC):
                    mp, s = divmod(mo, 2)
                    if s == 0:
                        if mp in wo_tiles:
                            ws = wo_tiles.pop(mp)
                        else:
                            ws = wop2.tile([128, 2, HC, 128], F8, tag="wo2")
                            nc.scalar.dma_start(
                                ws[:].opt(),
                                wo[2 * mp:2 * mp + 2].transpose([1, 0, 2]))
                        cur_wo = ws
                    ps = pso2_p.tile([128, TPC], F32, tag="o2")
                    for jp in range(8):
                        nc.tensor.matmul(
                            ps[:], cur_wo[:, s, 2 * jp:2 * jp + 2, :],
                            at_all[:, 2 * jp:2 * jp + 2, :],
                            start=(jp == 0), stop=(jp == 7), perf_mode=DR)
                    # x2 = psum * 2^-16 + x  (wo 1024x, at 64x)
                    nc.vector.scalar_tensor_tensor(
                        x2_sb[:, mo, :], ps[:], 2.0 ** -16, x16[:, mo, :],
                        MULT, ADD)
                    sq2 = lnD.tile([128, TPC], F16, tag="sq2")
                    nc.gpsimd.tensor_mul(sq2[:], x2_sb[:, mo, :],
                                         x2_sb[:, mo, :])
                    sq2_tiles[mo] = sq2
                    if mo >= LAG2:
                        stats2(mo - LAG2)
                for mo in range(HC - LAG2, HC):
                    stats2(mo)

                def put_h2(j, ps_c1, ps_c0):
                    t1 = lnD.tile([128, TPC], F16, tag="t1")
                    c0s = lnD.tile([128, TPC], F16, tag="c0s")
                    h2f = lnD.tile([128, TPC], F16, tag="h2f")
                    nc.vector.tensor_mul(t1[:], x2_sb[:, j, :], ps_c1[:])
                    nc.scalar.activation(c0s[:], ps_c0[:], AF.Copy)
                    nc.gpsimd.tensor_add(h2f[:], t1[:], c0s[:])
                    if j % 2 == 0:
                        nc.scalar.activation(h2x[:, j, 0, :], h2f[:], AF.Copy)
                    else:
                        nc.vector.tensor_copy(h2x[:, j, 0, :], h2f[:])
                    nc.gpsimd.tensor_sub(h2x[:, j, 1, :], h2f[:],
                                         h2x[:, j, 0, :])

                _ln_coeff(ps_sx2, ps_sq2, g2b2_sb, put_h2, lnD, psbcD)

              # ------------- Stage E: MLP -------------------------------------
              with (
                tc.tile_pool(name="wf1p", bufs=3) as wf1p,
                tc.tile_pool(name="wf2p", bufs=3) as wf2p,
                tc.tile_pool(name="mlt", bufs=4) as mlt,
                tc.tile_pool(name="psf1", bufs=3, space="PSUM") as psf1_p,
                tc.tile_pool(name="psf2", bufs=3, space="PSUM") as psf2_p,
              ):
                wf1_tiles = {}

                def wf1_load(mp):
                    ws = wf1p.tile([128, 2, HC, 2, 128], F8, tag="wf1")
                    nc.scalar.dma_start(
                        ws[:].opt(),
                        wf1[2 * mp:2 * mp + 2].transpose([1, 0, 2]))
                    wf1_tiles[mp] = ws

                wf1_load(0)
                wf1_load(1)
                for mo in range(FFC):
                    mp, s = divmod(mo, 2)
                    if s == 0:
                        cur_wf1 = wf1_tiles.pop(mp)
                        if 2 * (mp + 2) < FFC:
                            wf1_load(mp + 2)
                    ps = psf1_p.tile([128, TPC], F32, tag="f1")
                    for jp in range(8):  # hi*hi
                        nc.tensor.matmul(
                            ps[:], cur_wf1[:, s, 2 * jp:2 * jp + 2, 1, :],
                            h2x[:, 2 * jp:2 * jp + 2, 0, :],
                            start=(jp == 0), stop=False, perf_mode=DR)
                    for j in range(HC):  # cross: Wl*Ah + Wh*Al
                        nc.tensor.matmul(
                            ps[:], cur_wf1[:, s, j, :, :], h2x[:, j, :, :],
                            start=False, stop=(j == HC - 1), perf_mode=DR)
                    ff16 = mlt.tile([128, TPC], F16, tag="ff16")
                    nc.scalar.activation(ff16[:], ps[:], AF.Gelu,
                                         scale=1.0 / WS)
                    eng, oeng = ((nc.gpsimd, nc.vector) if mo % 2 == 0
                                 else (nc.vector, nc.gpsimd))
                    eng.tensor_copy(ffx[:, mo, 0, :], ff16[:])
                    oeng.tensor_sub(ffx[:, mo, 1, :], ff16[:],
                                    ffx[:, mo, 0, :])
                out_pair = [None]
                for mo in range(HC):
                    ws = wf2p.tile([128, FFC, 2, 128], F8, tag="wf2")
                    nc.scalar.dma_start(ws[:].opt(), wf2[mo])
                    ps = psf2_p.tile([128, TPC], F32, tag="f2")
                    for jp in range(FFC // 2):  # hi*hi
                        nc.tensor.matmul(
                            ps[:], ws[:, 2 * jp:2 * jp + 2, 1, :],
                            ffx[:, 2 * jp:2 * jp + 2, 0, :],
                            start=(jp == 0), stop=False, perf_mode=DR)
                    for j in range(FFC):  # cross
                        nc.tensor.matmul(
                            ps[:], ws[:, j, :, :], ffx[:, j, :, :],
                            start=False, stop=(j == FFC - 1), perf_mode=DR)
                    if mo % 2 == 0:
                        out_pair[0] = mlt.tile([128, 2, TPC], F32, tag="ot",
                                               bufs=2, name="otp")
                    ot = out_pair[0]
                    nc.vector.scalar_tensor_tensor(
                        ot[:, mo % 2, :], ps[:], 1.0 / WS, x2_sb[:, mo, :],
                        MULT, ADD)
                    if mo % 2 == 1:
                        nc.sync.dma_start(
                            outT[mo - 1:mo + 1, :, :].transpose([1, 0, 2]),
                            ot[:])
    return nc


def _build():
    if "nc" in _CACHE:
        return _CACHE["nc"]
    nc = bacc.Bacc(
        "TRN2", target_bir_lowering=False, debug=False,
        enable_asserts=True, num_devices=NCORES,
    )
    _emit(nc)
    nc.compile()
    _CACHE["nc"] = nc
    return nc


def _q8(v):
    return np.asarray(v, np.float32).astype(E4)


def prepare_inputs(x, pe, w_qkv, w_out, w_fc1, w_fc2, g1, b1, g2, b2):
    x = np.asarray(x, np.float32)
    pe = np.asarray(pe, np.float32)
    w_qkv = np.asarray(w_qkv, np.float32)
    w_out = np.asarray(w_out, np.float32)
    w_fc1 = np.asarray(w_fc1, np.float32)
    w_fc2 = np.asarray(w_fc2, np.float32)

    xf = x.reshape(TOK, HID)
    ropeC = np.tile(pe[:, 0::2].T, (2, B)).astype(np.float16)   # [128, TOK]
    ropeS = np.tile(pe[:, 1::2].T, (2, B)).astype(np.float16)
    g1b1 = np.stack([np.asarray(g1, np.float32),
                     np.asarray(b1, np.float32)]).astype(np.float16)
    g2b2 = np.stack([np.asarray(g2, np.float32),
                     np.asarray(b2, np.float32)]).astype(np.float16)

    # out_proj: strips over reordered contraction chunks
    # jj<8 -> head 2*jj (from a2a[0]); jj>=8 -> head 2*(jj-8)+1 (a2a[1])
    head_of = [2 * jj if jj < 8 else 2 * (jj - 8) + 1 for jj in range(HC)]
    wo_h = np.empty((HC, 128, HC * 128), dtype=E4)
    wt = (w_out * WS).astype(np.float32)   # [out, feat]
    for mo in range(HC):
        blk = np.empty((128, HC, 128), np.float32)
        for jj in range(HC):
            h = head_of[jj]
            blk[:, jj, :] = wt[mo * 128:(mo + 1) * 128,
                               h * 128:(h + 1) * 128].T
        wo_h[mo] = _q8(blk.reshape(128, HC * 128))

    def hilo_strips(w, n_strips, kc):
        w = (w * WS).astype(np.float32)
        hi = _q8(w).astype(np.float32)
        lo = _q8(w - hi).astype(np.float32)
        out = np.empty((n_strips, 128, kc * 2 * 128), dtype=E4)
        for mo in range(n_strips):
            rows = slice(mo * 128, (mo + 1) * 128)
            blk = np.empty((128, kc, 2, 128), np.float32)
            wl = lo[rows]; wh = hi[rows]    # [128(col), kc*128]
            blk[:, :, 0, :] = wl.reshape(128, kc, 128).transpose(2, 1, 0)
            blk[:, :, 1, :] = wh.reshape(128, kc, 128).transpose(2, 1, 0)
            out[mo] = _q8(blk.reshape(128, kc * 2 * 128))
        return out

    wf1_h = hilo_strips(w_fc1, FFC, HC)     # [FFC, 128, HC*2*128]
    wf2_h = hilo_strips(w_fc2, HC, FFC)     # [HC, 128, FFC*2*128]

    in_maps = []
    for c in range(NCORES):
        hsl = slice(2 * c * D, (2 * c + 2) * D)

        def qk_lay(rows):
            # rows [256, HID] (2 heads) -> [128(p), HC(j), 2(ch), 128(m,pp)]
            r = (rows * WS).astype(np.float32)
            t = r.reshape(2, 64, 2, HC, 128)       # [m, pp, ch, j, p]
            t = t.transpose(4, 3, 2, 0, 1)          # [p, j, ch, m, pp]
            return _q8(t.reshape(128, HC * 2 * 128))

        qrows = w_qkv[hsl]
        krows = w_qkv[HID + 2 * c * D: HID + (2 * c + 2) * D]
        vrows = w_qkv[2 * HID + 2 * c * D: 2 * HID + (2 * c + 2) * D]
        vv = (vrows * WS).astype(np.float32).reshape(2, 128, HC, 128)
        wv_c = _q8(vv.transpose(3, 2, 0, 1).reshape(128, HC * 256))

        xTc = np.ascontiguousarray(
            xf[c * TPC:(c + 1) * TPC].T).astype(np.float16)
        in_maps.append({
            "xT": xTc.reshape(HC, 128, TPC),
            "wq": qk_lay(qrows), "wk": qk_lay(krows), "wv": wv_c,
            "wo": wo_h, "wf1": wf1_h, "wf2": wf2_h,
            "g1b1": g1b1, "g2b2": g2b2,
            "ropeC": ropeC, "ropeS": ropeS,
        })
    return in_maps


def run(in_maps, **kwargs):
    nc = _build()
    return bass_utils.run_bass_kernel_spmd(
        nc, in_maps, core_ids=list(range(NCORES)), **kwargs
    )


def kernel(x, pe, w_qkv, w_out, w_fc1, w_fc2, g1, b1, g2, b2):
    in_maps = prepare_inputs(x, pe, w_qkv, w_out, w_fc1, w_fc2, g1, b1, g2, b2)
    res = run(in_maps)
    fullT = np.concatenate(
        [res.results[c]["outT"].reshape(HID, TPC) for c in range(NCORES)],
        axis=1)
    return np.ascontiguousarray(fullT.T).reshape(B, S, HID).astype(np.float32)


# revision 41
# speedup vs baseline: 1.6023x; 1.0025x over previous
"""MiniTransformerLayer on 8 Trainium2 NeuronCores — fp8 DoubleRow edition.

Sharding (as baseline): core c owns tokens [512c, 512(c+1)) and heads
{2c, 2c+1}; 2 AllGathers (LN1 out, fp8) + 2 AllToAlls (attn out, fp8).

Numerics:
  - all attention-side matmuls in fp8e4 DoubleRow (2 K-chunks per
    instruction, 0.5 cyc/row): qkv, scores (64-partition DR with heads
    stacked in partitions 0:64/64:128), attn@V, softmax denominator
    (ones-DR), out_proj.
  - MLP fc1/fc2 as 3-term hi/lo fp8 DR: W*A = Wh*Ah + (Wl*Ah + Wh*Al),
    weights hi/lo prepared on host, activations hi/lo on device.
  - softmax exp split across engines: Act native exp, plus a
    Schraudolph exp2 bit-trick pipeline (DVE f32->int32 convert from
    PSUM, Pool bitcast copy to fp8). The denominator cancels the shared
    approximation bias; measured end-to-end rel err ~4e-3.
  - LayerNorm stats via fp16 ones-matmuls; per-chunk scale/offset via
    rank-1/rank-2 coefficient matmuls (g,b folded).
  - fixed power-of-2 scales: weights x1024 (host), q/k/v fp8 at 16x,
    probs at 1x, attn out at 64x; unscaled in Act copies / stt epilogues.

Schedule notes: every dma_start costs ~630ns of serialized HWDGE
descriptor-generation, so transfers are batched through 3D DRAM
tensors + transposed access patterns (gathered h: 2 DMAs per token
block; attention outputs: 1 DMA per (head, batch); weights: 2 strips
per DMA). qkv for batch-1 token blocks is woven into the (head0,
batch0) attention blocks; LN stats matmuls lag their elementwise
producers to stay off the critical path.
"""

import sys

sys.path.insert(0, "/opt/trn_rl_repo")

import numpy as np
import ml_dtypes

import concourse.bass as bass
import concourse.bacc as bacc
import concourse.tile as tile
import concourse.mybir as mybir
from concourse import bass_utils

F8 = mybir.dt.float8e4
F16 = mybir.dt.float16
F32 = mybir.dt.float32
I32 = mybir.dt.int32
AF = mybir.ActivationFunctionType
DR = mybir.MatmulPerfMode.DoubleRow
E4 = ml_dtypes.float8_e4m3

NCORES = 8
B, S, HID, HEADS, D, FFN = 2, 2048, 2048, 16, 128, 4096
TOK = B * S            # 4096 flat tokens
TPC = TOK // NCORES    # 512 tokens per core
HC = HID // 128        # 16 hidden chunks
FFC = FFN // 128       # 32 ffn chunks
NH = HEADS // NCORES   # 2 heads per core
SCALE = 1.0 / float(np.sqrt(D))
EXP_BIAS = -3.0
EPS = 1e-5
WS = 1024.0            # host weight scale
QS = 2.0 ** -6         # psum(1024 q) -> 16 q
MULT, ADD, SUB = (mybir.AluOpType.mult, mybir.AluOpType.add,
                  mybir.AluOpType.subtract)
# Schraudolph exp2 bit trick: int32 bits = trunc(z*A + BC) ~ exp(z) bits
EXP_A = 12102203.161561485          # 2^23 / ln2
EXP_BC = 1064866805.0               # 127*2^23 - 366393
FE_SCALE = EXP_A * SCALE / 256.0
FE_BIAS = EXP_BC + EXP_A * EXP_BIAS

_CACHE = {}


def _emit(nc, single_core=False):
    xT = nc.dram_tensor("xT", [HC, 128, TPC], F16, kind="ExternalInput")
    wq = nc.dram_tensor("wq", [128, HC * 2 * 128], F8, kind="ExternalInput")
    wk = nc.dram_tensor("wk", [128, HC * 2 * 128], F8, kind="ExternalInput")
    wv = nc.dram_tensor("wv", [128, HC * 256], F8, kind="ExternalInput")
    wo = nc.dram_tensor("wo", [HC, 128, HC * 128], F8, kind="ExternalInput")
    wf1 = nc.dram_tensor("wf1", [FFC, 128, HC * 2 * 128], F8,
                         kind="ExternalInput")
    wf2 = nc.dram_tensor("wf2", [HC, 128, FFC * 2 * 128], F8,
                         kind="ExternalInput")
    g1b1 = nc.dram_tensor("g1b1", [2, HID], F16, kind="ExternalInput")
    g2b2 = nc.dram_tensor("g2b2", [2, HID], F16, kind="ExternalInput")
    ropeC = nc.dram_tensor("ropeC", [128, TOK], F16, kind="ExternalInput")
    ropeS = nc.dram_tensor("ropeS", [128, TOK], F16, kind="ExternalInput")
    outT = nc.dram_tensor("outT", [HC, 128, TPC], F32, kind="ExternalOutput")

    rg = [list(range(NCORES))]

    with tile.TileContext(nc) as tc:
        with (
            nc.allow_low_precision(reason="fp8 kernel: quantized by design"),
            tc.tile_pool(name="const", bufs=1) as const,
            tc.tile_pool(name="dram", bufs=1, space="DRAM") as dram,
            tc.tile_pool(name="wop", bufs=2) as wop,
        ):
            ones16 = const.tile([128, 1], F16, tag="on16")
            nc.vector.memset(ones16[:], 1.0)
            ones8q = const.tile([128, 2, 32], F8, tag="on8q")
            nc.vector.memset(ones8q[:], 0.25)
            eps_b = const.tile([1, 1], F32, tag="epsb")
            nc.vector.memset(eps_b[:], EPS)
            zero1_b = const.tile([1, 1], F32, tag="z1b")
            nc.vector.memset(zero1_b[:], 0.0)
            expb_b = const.tile([128, 1], F32, tag="expb")
            nc.vector.memset(expb_b[:], EXP_BIAS)
            g1b1_sb = const.tile([2, HID], F16, tag="g1b1")
            g2b2_sb = const.tile([2, HID], F16, tag="g2b2")
            nc.scalar.dma_start(g1b1_sb[:], g1b1[:])
            nc.scalar.dma_start(g2b2_sb[:], g2b2[:])

            # resident activations
            x16 = const.tile([128, HC, TPC], F16, tag="x16")
            q_sb = const.tile([128, 2, TOK], F8, tag="qsb")
            k_sb = const.tile([128, 2, TOK], F8, tag="ksb")
            v_sb = const.tile([128, TOK // 128, 256], F8, tag="vsb")
            at_all = const.tile([128, HC, TPC], F8, tag="atall")
            x2_sb = const.tile([128, HC, TPC], F16, tag="x2")

            # collective buffers: [slot/jj, partition, token] 3D layouts so
            # whole slots move in one descriptor-friendly DMA
            ag_in_a = dram.tile([8, 128, TPC], F8)
            ag_in_b = dram.tile([8, 128, TPC], F8)
            a2a_in_m = [dram.tile([NCORES, 128, TPC], F8, name=f"a2ai{m}")
                        for m in range(NH)]
            a2a_out_m = [dram.tile([NCORES, 128, TPC], F8, name=f"a2ao{m}")
                         for m in range(NH)]
            if single_core:
                ag_out_a = dram.tile([NCORES, 8, 128, TPC], F8)
                ag_out_b = dram.tile([NCORES, 8, 128, TPC], F8)
            else:
                ag_out_a = nc.dram_tensor(
                    "ag_out_a_sh", [NCORES, 8, 128, TPC], F8,
                    addr_space="Shared").ap()
                ag_out_b = nc.dram_tensor(
                    "ag_out_b_sh", [NCORES, 8, 128, TPC], F8,
                    addr_space="Shared").ap()

            def _ln_coeff(ps_sx, ps_sq, gb, put, lnp, psbc):
                mu = lnp.tile([1, TPC], F32, tag="mu", bufs=1)
                m2 = lnp.tile([1, TPC], F32, tag="m2", bufs=1)
                var = lnp.tile([1, TPC], F32, tag="var", bufs=1)
                lnv = lnp.tile([1, TPC], F32, tag="lnv", bufs=1)
                rstd16 = lnp.tile([1, TPC], F16, tag="rstd", bufs=1)
                mrs_ones = lnp.tile([2, TPC], F16, tag="mrso", bufs=1)
                nc.vector.tensor_scalar_mul(mu[:], ps_sx[:], 1.0 / HID)
                nc.vector.tensor_scalar_mul(m2[:], ps_sq[:], 1.0 / HID)
                nc.vector.tensor_mul(var[:], mu[:], mu[:])
                nc.vector.tensor_sub(var[:], m2[:], var[:])
                nc.scalar.activation(lnv[:], var[:], AF.Ln, bias=eps_b[:])
                nc.scalar.activation(rstd16[:], lnv[:], AF.Exp,
                                     bias=zero1_b[:], scale=-0.5)
                nc.vector.memset(mrs_ones[:], 1.0)
                nc.vector.tensor_mul(mrs_ones[0:1, :], mu[:], rstd16[:])
                nc.vector.tensor_scalar_mul(mrs_ones[0:1, :],
                                            mrs_ones[0:1, :], -1.0)
                for j in range(HC):
                    cs = slice(j * 128, (j + 1) * 128)
                    ps_c1 = psbc.tile([128, TPC], F32, tag="bc")
                    ps_c0 = psbc.tile([128, TPC], F32, tag="bc")
                    nc.tensor.matmul(ps_c1[:], gb[0:1, cs], rstd16[:],
                                     start=True, stop=True)
                    nc.tensor.matmul(ps_c0[:], gb[0:2, cs], mrs_ones[:],
                                     start=True, stop=True)
                    put(j, ps_c1, ps_c0)

            # ---------------- Stage A: load x, LN1, AllGather ----------------
            with (
                tc.tile_pool(name="lnA", bufs=4) as lnA,
                tc.tile_pool(name="psstA", bufs=2, space="PSUM") as psstA,
                tc.tile_pool(name="psbcA", bufs=4, space="PSUM") as psbcA,
            ):
                for g in range(4):
                    nc.sync.dma_start(
                        x16[:, 4 * g:4 * g + 4, :],
                        xT[4 * g:4 * g + 4, :, :].transpose([1, 0, 2]))
                ps_sx = psstA.tile([1, TPC], F32, tag="st")
                ps_sq = psstA.tile([1, TPC], F32, tag="st")
                sq_tiles = {}
                LAG = 3

                def stats1(j):
                    nc.tensor.matmul(ps_sx[:], ones16[:], x16[:, j, :],
                                     start=(j == 0), stop=(j == HC - 1))
                    nc.tensor.matmul(ps_sq[:], ones16[:], sq_tiles.pop(j)[:],
                                     start=(j == 0), stop=(j == HC - 1))

                for j in range(HC):
                    sqt = lnA.tile([128, TPC], F16, tag="sqt")
                    nc.vector.tensor_mul(sqt[:], x16[:, j, :], x16[:, j, :])
                    sq_tiles[j] = sqt
                    if j >= LAG:
                        stats1(j - LAG)
                for j in range(HC - LAG, HC):
                    stats1(j)

                h1_pair = [None]

                def put_h1(j, ps_c1, ps_c0):
                    # GPSIMD can't read PSUM: DVE does the psum mul, Act
                    # copies c0 to SBUF, Pool does the SBUF-only add.
                    t1 = lnA.tile([128, TPC], F16, tag="t1")
                    c0s = lnA.tile([128, TPC], F16, tag="c0s")
                    nc.vector.tensor_mul(t1[:], x16[:, j, :], ps_c1[:])
                    nc.scalar.activation(c0s[:], ps_c0[:], AF.Copy)
                    if j % 2 == 0:
                        h1_pair[0] = lnA.tile([128, 2, TPC], F8, tag="h1",
                                              bufs=2, name="h1p")
                    h1 = h1_pair[0]
                    nc.gpsimd.tensor_add(h1[:, j % 2, :], t1[:], c0s[:])
                    if j % 2 == 1:
                        u = j // 2
                        half, uu = (ag_in_a, u) if u < 4 else (ag_in_b, u - 4)
                        nc.sync.dma_start(
                            half[2 * uu:2 * uu + 2, :, :].transpose(
                                [1, 0, 2]), h1[:])

                _ln_coeff(ps_sx, ps_sq, g1b1_sb, put_h1, lnA, psbcA)

            if single_core:
                # AllGather stand-in: quarter volume per rank slot (baseline
                # convention) as every-4th-partition-row full-width writes —
                # 512B lines, one DMA per buffer, strided range still covers
                # every slot for dependency tracking.
                for r in range(0, NCORES, 4):
                    nc.sync.dma_start(
                        ag_out_a[r:r + 4, :, ::4, :],
                        ag_in_a[:, ::4, :].unsqueeze(0).to_broadcast(
                            (4, 8, 32, TPC)))
                    nc.sync.dma_start(
                        ag_out_b[r:r + 4, :, ::4, :],
                        ag_in_b[:, ::4, :].unsqueeze(0).to_broadcast(
                            (4, 8, 32, TPC)))
            else:
                nc.gpsimd.collective_compute(
                    "AllGather", mybir.AluOpType.bypass, replica_groups=rg,
                    ins=[ag_in_a.opt()], outs=[ag_out_a.opt()])
                nc.gpsimd.collective_compute(
                    "AllGather", mybir.AluOpType.bypass, replica_groups=rg,
                    ins=[ag_in_b.opt()], outs=[ag_out_b.opt()])

            # ---------------- Stages B+C: qkv + attention (woven) -------------
            SB = S // TPC   # 4 query blocks per batch
            with (
                tc.tile_pool(name="cp", bufs=4) as cp,
                tc.tile_pool(name="ptp", bufs=3) as ptp,
                tc.tile_pool(name="zip", bufs=2) as zip_,
            ):
                def attn_block(m, b, qb, fast_pairs, pools, at4):
                    """fast_pairs kg-indices go through the DVE(+Pool)
                    fast-exp pipeline, the rest through Act exp. den/pv
                    matmuls are emitted 2 pairs behind the scores so the
                    in-order PE never waits on an in-flight exp."""
                    pss_p, pso_p, psdn_p = pools
                    mr = slice(64 * m, 64 * (m + 1))
                    qsl = slice(b * S + qb * TPC, b * S + (qb + 1) * TPC)
                    pso = pso_p.tile([128, TPC], F32, tag="o")
                    psden = psdn_p.tile([32, TPC], F32, tag="dn")
                    pts = {}

                    def score_exp(kg, pool_copy):
                        pss = pss_p.tile([128, 2, TPC], F32, tag="s")
                        for h_ in range(2):
                            kc = kg * 2 + h_
                            ksl = slice(b * S + kc * 128,
                                        b * S + (kc + 1) * 128)
                            nc.tensor.matmul(
                                pss[:, h_, :], k_sb[mr, :, ksl],
                                q_sb[mr, :, qsl],
                                start=True, stop=True, perf_mode=DR)
                        pt = ptp.tile([128, 2, TPC], F8, tag="pt", bufs=6)
                        pts[kg] = pt
                        if kg in fast_pairs:
                            zi = zip_.tile([128, 2, TPC], I32, tag="ziw",
                                           bufs=3)
                            nc.vector.tensor_scalar(
                                zi[:].opt(), pss[:].opt(), FE_SCALE, FE_BIAS,
                                MULT, ADD)
                            ceng = nc.gpsimd if pool_copy else nc.vector
                            ceng.tensor_copy(pt[:].opt(),
                                             zi[:].opt().bitcast(F32))
                        else:
                            nc.scalar.activation(
                                pt[:].opt(), pss[:].opt(), AF.Exp,
                                scale=SCALE / 256.0, bias=expb_b[:])

                    def den_pv(kg):
                        pt = pts.pop(kg)
                        nc.tensor.matmul(
                            psden[:], ones8q[:], pt[:],
                            start=(kg == 0), stop=(kg == 7), perf_mode=DR)
                        nc.tensor.matmul(
                            pso[:],
                            v_sb[:, b * 16 + 2 * kg:b * 16 + 2 * kg + 2,
                                 m * 128:(m + 1) * 128],
                            pt[:],
                            start=(kg == 0), stop=(kg == 7), perf_mode=DR)

                    nf = 0
                    for kg in range(8):
                        if kg in fast_pairs:
                            nf += 1
                        score_exp(kg, pool_copy=(nf % 2 == 1))
                        if kg >= 2:
                            den_pv(kg - 2)
                    den_pv(6)
                    den_pv(7)
                    rec16 = cp.tile([1, TPC], F16, tag="rec")
                    nc.vector.reciprocal(rec16[:], psden[0:1, :])
                    rb = cp.tile([128, TPC], F16, tag="rb")
                    nc.gpsimd.partition_broadcast(rb[:], rec16[:])
                    nc.vector.tensor_mul(at4[:, qb, :], pso[:], rb[:])
                    if qb == SB - 1:
                        nc.sync.dma_start(
                            a2a_in_m[m][b * SB:(b + 1) * SB, :, :].transpose(
                                [1, 0, 2]), at4[:])

                def attn_two(m, b, qb0, fast_pairs, pools, at4):
                    """Interleave two query blocks so engines always have
                    independent work between dependency hops."""
                    pss_p, pso_p, psdn_p = pools
                    mr = slice(64 * m, 64 * (m + 1))
                    ctx = []
                    for qb in (qb0, qb0 + 1):
                        qsl = slice(b * S + qb * TPC, b * S + (qb + 1) * TPC)
                        pso = pso_p.tile([128, TPC], F32, tag="o")
                        psden = psdn_p.tile([32, TPC], F32, tag="dn")
                        ctx.append((qb, qsl, pso, psden, {}))

                    cnt = [0]

                    def score_exp(ci, kg):
                        # per-kc pss tiles (1 PSUM bank each) keep the
                        # scores->exp ring turning at full rate
                        qb, qsl, pso, psden, pts = ctx[ci]
                        pt = ptp.tile([128, 2, TPC], F8, tag="pt", bufs=6)
                        pts[kg] = pt
                        for h_ in range(2):
                            kc = kg * 2 + h_
                            ksl = slice(b * S + kc * 128,
                                        b * S + (kc + 1) * 128)
                            pss = pss_p.tile([128, TPC], F32, tag="s")
                            nc.tensor.matmul(
                                pss[:], k_sb[mr, :, ksl], q_sb[mr, :, qsl],
                                start=True, stop=True, perf_mode=DR)
                            cnt[0] += 1
                            if kg in fast_pairs:
                                zi = zip_.tile([128, TPC], I32, tag="zi",
                                               bufs=4)
                                nc.vector.tensor_scalar(
                                    zi[:], pss[:], FE_SCALE, FE_BIAS,
                                    MULT, ADD)
                                ceng = (nc.vector if cnt[0] % 4 == 0
                                        else nc.gpsimd)
                                ceng.tensor_copy(pt[:, h_, :],
                                                 zi[:].bitcast(F32))
                            else:
                                nc.scalar.activation(
                                    pt[:, h_, :], pss[:], AF.Exp,
                                    scale=SCALE / 256.0, bias=expb_b[:])

                    def den_pv(ci, kg):
                        qb, qsl, pso, psden, pts = ctx[ci]
                        pt = pts.pop(kg)
                        nc.tensor.matmul(
                            psden[:], ones8q[:], pt[:],
                            start=(kg == 0), stop=(kg == 7), perf_mode=DR)
                        nc.tensor.matmul(
                            pso[:],
                            v_sb[:, b * 16 + 2 * kg:b * 16 + 2 * kg + 2,
                                 m * 128:(m + 1) * 128],
                            pt[:],
                            start=(kg == 0), stop=(kg == 7), perf_mode=DR)

                    for kg in range(8):
                        for ci in range(2):
                            score_exp(ci, kg)
                        if kg >= 2:
                            den_pv(0, kg - 2)
                            den_pv(1, kg - 2)
                    for kg in (6, 7):
                        den_pv(0, kg)
                        den_pv(1, kg)
                    for ci in range(2):
                        qb, qsl, pso, psden, pts = ctx[ci]
                        rec16 = cp.tile([1, TPC], F16, tag="rec")
                        nc.vector.reciprocal(rec16[:], psden[0:1, :])
                        rb = cp.tile([128, TPC], F16, tag="rb")
                        nc.gpsimd.partition_broadcast(rb[:], rec16[:])
                        nc.vector.tensor_mul(at4[:, qb, :], pso[:], rb[:])
                        if qb == SB - 1:
                            nc.sync.dma_start(
                                a2a_in_m[m][b * SB:(b + 1) * SB, :, :]
                                .transpose([1, 0, 2]), at4[:])

                def a2a(m):
                    if single_core:
                        # two-hop stand-in (baseline convention), halves
                        # pipelined so wire transfers overlap
                        a2a_mid = dram.tile([NCORES, 128, TPC], F8,
                                            name=f"a2am{m}")
                        for hh in range(2):
                            sl = slice(4 * hh, 4 * hh + 4)
                            nc.sync.dma_start(a2a_mid[sl].opt(),
                                              a2a_in_m[m][sl].opt())
                        for hh in range(2):
                            sl = slice(4 * hh, 4 * hh + 4)
                            nc.sync.dma_start(a2a_out_m[m][sl].opt(),
                                              a2a_mid[sl].opt())
                    else:
                        nc.gpsimd.collective_compute(
                            "AllToAll", mybir.AluOpType.bypass,
                            replica_groups=rg,
                            ins=[a2a_in_m[m].opt()], outs=[a2a_out_m[m].opt()])

                with (
                    tc.tile_pool(name="bwt", bufs=1) as bwt,
                    tc.tile_pool(name="htc", bufs=4) as htc,
                    tc.tile_pool(name="qkpre", bufs=2) as qkpre,
                    tc.tile_pool(name="ropet", bufs=4) as ropet,
                    tc.tile_pool(name="pssA", bufs=2, space="PSUM") as pssA,
                    tc.tile_pool(name="psoA", bufs=1, space="PSUM") as psoA,
                    tc.tile_pool(name="psdnA", bufs=1, space="PSUM") as psdnA,
                    tc.tile_pool(name="psqk", bufs=1, space="PSUM") as psqk,
                    tc.tile_pool(name="psv", bufs=1, space="PSUM") as psv,
                ):
                    poolsA = (pssA, psoA, psdnA)
                    rC = bwt.tile([128, TOK], F16, tag="rC")
                    rS = bwt.tile([128, TOK], F16, tag="rS")
                    nc.scalar.dma_start(rC[:], ropeC[:])
                    nc.scalar.dma_start(rS[:], ropeS[:])
                    wq_sb = bwt.tile([128, HC, 2, 128], F8, tag="wq")
                    wk_sb = bwt.tile([128, HC, 2, 128], F8, tag="wk")
                    wv_sb = bwt.tile([128, HC, 256], F8, tag="wv")
                    nc.scalar.dma_start(wq_sb[:].opt(), wq[:])
                    nc.scalar.dma_start(wk_sb[:].opt(), wk[:])
                    nc.scalar.dma_start(wv_sb[:].opt(), wv[:])

                    def qkv_tb(tb):
                        # gathered h for this token block: one DMA per half
                        ta = htc.tile([128, 8, TPC], F8, tag="hta")
                        tb_ = htc.tile([128, 8, TPC], F8, tag="htb")
                        nc.sync.dma_start(
                            ta[:], ag_out_a[tb].transpose([1, 0, 2]))
                        nc.sync.dma_start(
                            tb_[:], ag_out_b[tb].transpose([1, 0, 2]))
                        hts = ([ta[:, 2 * u:2 * u + 2, :] for u in range(4)]
                               + [tb_[:, 2 * u:2 * u + 2, :]
                                  for u in range(4)])
                        tcols = slice(tb * TPC, (tb + 1) * TPC)
                        for (w_sb, dst) in ((wq_sb, q_sb), (wk_sb, k_sb)):
                            pre = qkpre.tile([128, 2, TPC], F16, tag="pre")
                            for ch in range(2):
                                ps = psqk.tile([128, TPC], F32, tag="qk")
                                for jp in range(8):
                                    nc.tensor.matmul(
                                        ps[:],
                                        w_sb[:, 2 * jp:2 * jp + 2, ch, :],
                                        hts[jp], start=(jp == 0),
                                        stop=(jp == 7), perf_mode=DR)
                                nc.scalar.activation(pre[:, ch, :], ps[:],
                                                     AF.Copy, scale=QS)
                            # RoPE: 4 muls on DVE, 2 fp8 combines on Pool
                            t1 = ropet.tile([128, TPC], F16, tag="t1")
                            t2 = ropet.tile([128, TPC], F16, tag="t2")
                            t3 = ropet.tile([128, TPC], F16, tag="t3")
                            t4 = ropet.tile([128, TPC], F16, tag="t4")
                            nc.vector.tensor_mul(t1[:], pre[:, 0, :],
                                                 rC[:, tcols])
                            nc.vector.tensor_mul(t2[:], pre[:, 1, :],
                                                 rS[:, tcols])
                            nc.vector.tensor_mul(t3[:], pre[:, 0, :],
                                                 rS[:, tcols])
                            nc.vector.tensor_mul(t4[:], pre[:, 1, :],
                                                 rC[:, tcols])
                            nc.gpsimd.tensor_sub(dst[:, 0, tcols], t1[:],
                                                 t2[:])
                            nc.gpsimd.tensor_add(dst[:, 1, tcols], t3[:],
                                                 t4[:])
                        for mt in range(4):
                            ps = psv.tile([128, 256], F32, tag="v")
                            for jp in range(8):
                                nc.tensor.matmul(
                                    ps[:],
                                    hts[jp][:, :, mt * 128:(mt + 1) * 128],
                                    wv_sb[:, 2 * jp:2 * jp + 2, :],
                                    start=(jp == 0), stop=(jp == 7),
                                    perf_mode=DR)
                            nc.scalar.activation(v_sb[:, tb * 4 + mt, :],
                                                 ps[:], AF.Copy, scale=QS)

                    for tb in range(4):
                        qkv_tb(tb)
                    # weave: batch-0 attention of head 0 x qkv for batch 1.
                    at4 = cp.tile([128, SB, TPC], F8, tag="at4", bufs=2)
                    for qb in range(SB):
                        attn_block(0, 0, qb, {2, 5}, poolsA, at4)
                        qkv_tb(4 + qb)
                # qkv pools closed: wider psum rings for remaining attention
                with (
                    tc.tile_pool(name="pssB", bufs=4, space="PSUM") as pssB,
                    tc.tile_pool(name="psoB", bufs=2, space="PSUM") as psoB,
                    tc.tile_pool(name="psdnB", bufs=2, space="PSUM") as psdnB,
                ):
                    poolsB = (pssB, psoB, psdnB)
                    at4 = cp.tile([128, SB, TPC], F8, tag="at4", bufs=2)
                    for qb in (0, 2):
                        attn_two(0, 1, qb, {1, 3, 5}, poolsB, at4)
                    a2a(0)
                    # prefetch: even at_all chunks + first wo pairs during m1
                    for hh in range(2):
                        nc.sync.dma_start(
                            at_all[:, 4 * hh:4 * hh + 4, :],
                            a2a_out_m[0][4 * hh:4 * hh + 4].transpose(
                                [1, 0, 2]))
                    wo_tiles = {}
                    for mp in range(2):
                        ws = wop.tile([128, 2, HC, 128], F8, tag="wo")
                        nc.scalar.dma_start(
                            ws[:].opt(),
                            wo[2 * mp:2 * mp + 2].transpose([1, 0, 2]))
                        wo_tiles[mp] = ws
                    for b in range(B):
                        at4 = cp.tile([128, SB, TPC], F8, tag="at4", bufs=2)
                        for qb in (0, 2):
                            attn_two(1, b, qb, {1, 3, 5}, poolsB, at4)
                    a2a(1)

            # ------------- Stages D+E ----------------------------------------
            with tc.tile_pool(name="late", bufs=1) as late:
              h2x = late.tile([128, HC, 2, TPC], F8, tag="h2x")
              ffx = late.tile([128, FFC, 2, TPC], F8, tag="ffx")
              with (
                tc.tile_pool(name="wop2", bufs=3) as wop2,
                tc.tile_pool(name="lnD", bufs=4) as lnD,
                tc.tile_pool(name="pso2", bufs=3, space="PSUM") as pso2_p,
                tc.tile_pool(name="psstD", bufs=2, space="PSUM") as psstD,
                tc.tile_pool(name="psbcD", bufs=3, space="PSUM") as psbcD,
              ):
                for hh in range(2):
                    nc.sync.dma_start(
                        at_all[:, 8 + 4 * hh:12 + 4 * hh, :],
                        a2a_out_m[1][4 * hh:4 * hh + 4].transpose([1, 0, 2]))
                ps_sx2 = psstD.tile([1, TPC], F32, tag="st")
                ps_sq2 = psstD.tile([1, TPC], F32, tag="st")
                sq2_tiles = {}
                LAG2 = 2

                def stats2(mo):
                    nc.tensor.matmul(ps_sx2[:], ones16[:], x2_sb[:, mo, :],
                                     start=(mo == 0), stop=(mo == HC - 1))
                    nc.tensor.matmul(ps_sq2[:], ones16[:],
                                     sq2_tiles.pop(mo)[:],
                                     start=(mo == 0), stop=(mo == HC - 1))

                for mo in range(HC):
                    mp, s = divmod(mo, 2)
                    if s == 0:
                        if mp in wo_tiles:
                            ws = wo_tiles.pop(mp)
                        else:
                            ws = wop2.tile([128, 2, HC, 128], F8, tag="wo2")
                            nc.scalar.dma_start(
                                ws[:].opt(),
                                wo[2 * mp:2 * mp + 2].transpose([1, 0, 2]))
                        cur_wo = ws
                    ps = pso2_p.tile([128, TPC], F32, tag="o2")
                    for jp in range(8):
                        nc.tensor.matmul(
                            ps[:], cur_wo[:, s, 2 * jp:2 * jp + 2, :],
                            at_all[:, 2 * jp:2 * jp + 2, :],
                            start=(jp == 0), stop=(jp == 7), perf_mode=DR)
                    # x2 = psum * 2^-16 + x  (wo 1024x, at 64x)
                    nc.vector.scalar_tensor_tensor(
                        x2_sb[:, mo, :], ps[:], 2.0 ** -16, x16[:, mo, :],
                        MULT, ADD)
                    sq2 = lnD.tile([128, TPC], F16, tag="sq2")
                    nc.vector.tensor_mul(sq2[:], x2_sb[:, mo, :],
                                         x2_sb[:, mo, :])
                    sq2_tiles[mo] = sq2
                    if mo >= LAG2:
                        stats2(mo - LAG2)
                for mo in range(HC - LAG2, HC):
                    stats2(mo)

                def put_h2(j, ps_c1, ps_c0):
                    t1 = lnD.tile([128, TPC], F16, tag="t1")
                    c0s = lnD.tile([128, TPC], F16, tag="c0s")
                    h2f = lnD.tile([128, TPC], F16, tag="h2f")
                    nc.vector.tensor_mul(t1[:], x2_sb[:, j, :], ps_c1[:])
                    nc.scalar.activation(c0s[:], ps_c0[:], AF.Copy)
                    nc.gpsimd.tensor_add(h2f[:], t1[:], c0s[:])
                    if j % 2 == 0:
                        nc.scalar.activation(h2x[:, j, 0, :], h2f[:], AF.Copy)
                    else:
                        nc.vector.tensor_copy(h2x[:, j, 0, :], h2f[:])
                    nc.gpsimd.tensor_sub(h2x[:, j, 1, :], h2f[:],
                                         h2x[:, j, 0, :])

                _ln_coeff(ps_sx2, ps_sq2, g2b2_sb, put_h2, lnD, psbcD)

              # ------------- Stage E: MLP -------------------------------------
              with (
                tc.tile_pool(name="wf1p", bufs=3) as wf1p,
                tc.tile_pool(name="wf2p", bufs=3) as wf2p,
                tc.tile_pool(name="mlt", bufs=4) as mlt,
                tc.tile_pool(name="psf1", bufs=3, space="PSUM") as psf1_p,
                tc.tile_pool(name="psf2", bufs=3, space="PSUM") as psf2_p,
              ):
                wf1_tiles = {}

                def wf1_load(mp):
                    ws = wf1p.tile([128, 2, HC, 2, 128], F8, tag="wf1")
                    nc.scalar.dma_start(
                        ws[:].opt(),
                        wf1[2 * mp:2 * mp + 2].transpose([1, 0, 2]))
                    wf1_tiles[mp] = ws

                wf1_load(0)
                wf1_load(1)
                for mo in range(FFC):
                    mp, s = divmod(mo, 2)
                    if s == 0:
                        cur_wf1 = wf1_tiles.pop(mp)
                        if 2 * (mp + 2) < FFC:
                            wf1_load(mp + 2)
                    ps = psf1_p.tile([128, TPC], F32, tag="f1")
                    for jp in range(8):  # hi*hi
                        nc.tensor.matmul(
                            ps[:], cur_wf1[:, s, 2 * jp:2 * jp + 2, 1, :],
                            h2x[:, 2 * jp:2 * jp + 2, 0, :],
                            start=(jp == 0), stop=False, perf_mode=DR)
                    for j in range(HC):  # cross: Wl*Ah + Wh*Al
                        nc.tensor.matmul(
                            ps[:], cur_wf1[:, s, j, :, :], h2x[:, j, :, :],
                            start=False, stop=(j == HC - 1), perf_mode=DR)
                    ff16 = mlt.tile([128, TPC], F16, tag="ff16")
                    nc.scalar.activation(ff16[:], ps[:], AF.Gelu,
                                         scale=1.0 / WS)
                    eng, oeng = ((nc.gpsimd, nc.vector) if mo % 2 == 0
                                 else (nc.vector, nc.gpsimd))
                    eng.tensor_copy(ffx[:, mo, 0, :], ff16[:])
                    oeng.tensor_sub(ffx[:, mo, 1, :], ff16[:],
                                    ffx[:, mo, 0, :])
                out_pair = [None]
                for mo in range(HC):
                    ws = wf2p.tile([128, FFC, 2, 128], F8, tag="wf2")
                    nc.scalar.dma_start(ws[:].opt(), wf2[mo])
                    ps = psf2_p.tile([128, TPC], F32, tag="f2")
                    for jp in range(FFC // 2):  # hi*hi
                        nc.tensor.matmul(
                            ps[:], ws[:, 2 * jp:2 * jp + 2, 1, :],
                            ffx[:, 2 * jp:2 * jp + 2, 0, :],
                            start=(jp == 0), stop=False, perf_mode=DR)
                    for j in range(FFC):  # cross
                        nc.tensor.matmul(
                            ps[:], ws[:, j, :, :], ffx[:, j, :, :],
                            start=False, stop=(j == FFC - 1), perf_mode=DR)
                    if mo % 2 == 0:
                        out_pair[0] = mlt.tile([128, 2, TPC], F32, tag="ot",
                                               bufs=2, name="otp")
                    ot = out_pair[0]
                    nc.vector.scalar_tensor_tensor(
                        ot[:, mo % 2, :], ps[:], 1.0 / WS, x2_sb[:, mo, :],
                        MULT, ADD)
                    if mo == HC - 1:
                        # last pair as two singles: shorter drain tail
                        nc.sync.dma_start(
                            outT[mo - 1, :, :], ot[:, 0, :])
                        nc.sync.dma_start(
                            outT[mo, :, :], ot[:, 1, :])
                    elif mo % 2 == 1:
                        nc.sync.dma_start(
                            outT[mo - 1:mo + 1, :, :].transpose([1, 0, 2]),
                            ot[:])
    return nc


def _build():
    if "nc" in _CACHE:
        return _CACHE["nc"]
    nc = bacc.Bacc(
        "TRN2", target_bir_lowering=False, debug=False,
        enable_asserts=True, num_devices=NCORES,
    )
    _emit(nc)
    nc.compile()
    _CACHE["nc"] = nc
    return nc


def _q8(v):
    return np.asarray(v, np.float32).astype(E4)


def prepare_inputs(x, pe, w_qkv, w_out, w_fc1, w_fc2, g1, b1, g2, b2):
    x = np.asarray(x, np.float32)
    pe = np.asarray(pe, np.float32)
    w_qkv = np.asarray(w_qkv, np.float32)
    w_out = np.asarray(w_out, np.float32)
    w_fc1 = np.asarray(w_fc1, np.float32)
    w_fc2 = np.asarray(w_fc2, np.float32)

    xf = x.reshape(TOK, HID)
    ropeC = np.tile(pe[:, 0::2].T, (2, B)).astype(np.float16)   # [128, TOK]
    ropeS = np.tile(pe[:, 1::2].T, (2, B)).astype(np.float16)
    g1b1 = np.stack([np.asarray(g1, np.float32),
                     np.asarray(b1, np.float32)]).astype(np.float16)
    g2b2 = np.stack([np.asarray(g2, np.float32),
                     np.asarray(b2, np.float32)]).astype(np.float16)

    # out_proj: strips over reordered contraction chunks
    # jj<8 -> head 2*jj (from a2a[0]); jj>=8 -> head 2*(jj-8)+1 (a2a[1])
    head_of = [2 * jj if jj < 8 else 2 * (jj - 8) + 1 for jj in range(HC)]
    wo_h = np.empty((HC, 128, HC * 128), dtype=E4)
    wt = (w_out * WS).astype(np.float32)   # [out, feat]
    for mo in range(HC):
        blk = np.empty((128, HC, 128), np.float32)
        for jj in range(HC):
            h = head_of[jj]
            blk[:, jj, :] = wt[mo * 128:(mo + 1) * 128,
                               h * 128:(h + 1) * 128].T
        wo_h[mo] = _q8(blk.reshape(128, HC * 128))

    def hilo_strips(w, n_strips, kc):
        w = (w * WS).astype(np.float32)
        hi = _q8(w).astype(np.float32)
        lo = _q8(w - hi).astype(np.float32)
        out = np.empty((n_strips, 128, kc * 2 * 128), dtype=E4)
        for mo in range(n_strips):
            rows = slice(mo * 128, (mo + 1) * 128)
            blk = np.empty((128, kc, 2, 128), np.float32)
            wl = lo[rows]; wh = hi[rows]    # [128(col), kc*128]
            blk[:, :, 0, :] = wl.reshape(128, kc, 128).transpose(2, 1, 0)
            blk[:, :, 1, :] = wh.reshape(128, kc, 128).transpose(2, 1, 0)
            out[mo] = _q8(blk.reshape(128, kc * 2 * 128))
        return out

    wf1_h = hilo_strips(w_fc1, FFC, HC)     # [FFC, 128, HC*2*128]
    wf2_h = hilo_strips(w_fc2, HC, FFC)     # [HC, 128, FFC*2*128]

    in_maps = []
    for c in range(NCORES):
        hsl = slice(2 * c * D, (2 * c + 2) * D)

        def qk_lay(rows):
            # rows [256, HID] (2 heads) -> [128(p), HC(j), 2(ch), 128(m,pp)]
            r = (rows * WS).astype(np.float32)
            t = r.reshape(2, 64, 2, HC, 128)       # [m, pp, ch, j, p]
            t = t.transpose(4, 3, 2, 0, 1)          # [p, j, ch, m, pp]
            return _q8(t.reshape(128, HC * 2 * 128))

        qrows = w_qkv[hsl]
        krows = w_qkv[HID + 2 * c * D: HID + (2 * c + 2) * D]
        vrows = w_qkv[2 * HID + 2 * c * D: 2 * HID + (2 * c + 2) * D]
        vv = (vrows * WS).astype(np.float32).reshape(2, 128, HC, 128)
        wv_c = _q8(vv.transpose(3, 2, 0, 1).reshape(128, HC * 256))

        xTc = np.ascontiguousarray(
            xf[c * TPC:(c + 1) * TPC].T).astype(np.float16)
        in_maps.append({
            "xT": xTc.reshape(HC, 128, TPC),
            "wq": qk_lay(qrows), "wk": qk_lay(krows), "wv": wv_c,
            "wo": wo_h, "wf1": wf1_h, "wf2": wf2_h,
            "g1b1": g1b1, "g2b2": g2b2,
            "ropeC": ropeC, "ropeS": ropeS,
        })
    return in_maps


def run(in_maps, **kwargs):
    nc = _build()
    return bass_utils.run_bass_kernel_spmd(
        nc, in_maps, core_ids=list(range(NCORES)), **kwargs
    )


def kernel(x, pe, w_qkv, w_out, w_fc1, w_fc2, g1, b1, g2, b2):
    in_maps = prepare_inputs(x, pe, w_qkv, w_out, w_fc1, w_fc2, g1, b1, g2, b2)
    res = run(in_maps)
    fullT = np.concatenate(
        [res.results[c]["outT"].reshape(HID, TPC) for c in range(NCORES)],
        axis=1)
    return np.ascontiguousarray(fullT.T).reshape(B, S, HID).astype(np.float32)
